# revision 11
# baseline (speedup 1.0000x reference)
"""Trainium2 Bass kernel for nn_CausalTrajectoryPrediction.

Math (per node n of 64, batch B=1024):
    h1 = relu(x_masked @ W1[n].T)          x_masked = x with col n zeroed
    r1 = relu(h1 @ W2[n].T)
    h3 = relu([r1, x_n] @ W3[n].T + b3[n])
    out[:, n] = relu(h3 @ W4[n] + b4[n])

Restructuring (validated vs the fp32 reference on CPU):
  - The input mask folds into the weights on the host (zero the diagonal
    column of W1[n]); the "own value" path of W3 collapses to one column;
    b3 becomes a ones-row of the layer-3 stationary operand.
  - The last layer is eliminated via w*relu(z) = 0.5*(w*z + w*|z|):
        out[:, n] = 0.5 * relu(c_pos - c_neg)
    where c_pos/c_neg are DVE abs-reduces over two fixed windows of the
    layer-3 PSUM row. Columns are pre-scaled by |W4| and grouped by
    sign(W4) on the host. The linear term a = rep @ (W3ext@W4 + 2*b4*e1)
    is carried by two extra nonnegative columns (v+ in the positive
    window, v- in the negative window; rep is made elementwise >= 0 by
    splitting x_n into x+/x-), so no extra matmul or PSUM tile is needed.
  - All 8 cores share one program, but the sign-split point differs per
    node. Nodes are assigned to program slots sorted by split point; the
    few "middle" columns that are positive on some cores and negative on
    others sit inside the positive window and are duplicated at the tail
    with weight 2 (|z| - 2|z| = -|z|) on cores where they are negative.
  - Sharding: 8 nodes per core (weights are NOT replicated -> 8x less
    HBM traffic), full batch per core. Host gathers (1024, 8) per core.

v3 changes vs v2 (trace-driven, see HW model below):
  - DMA prefetch: every transfer now reads a CONTIGUOUS DRAM tensor
    (v2 sliced wide tensors; the strided reads landed all descriptors on
    ONE of the 16 SDMA engines -> 25 GB/s, weights not resident until
    t=38us, PE stalling + HAM re-throttling to 1.2 GHz mid-kernel).
    Transfers are spread over three queues (sync/scalar HWDGE + gpsimd
    SWDGE - idle before compute starts) and ordered by first use.
  - Warm-up scratch memset moved to the (otherwise idle) Vector engine
    and the warm psum to the ps3 pool, so the dummy-matmul burst starts
    right after the engine preamble (~5us) instead of ~8us, and the PE's
    HAM clock gate (default K=4/8 = 1.2 GHz; K=8/8 = 2.4 GHz after
    ~3.4us of sustained busy) is fully open when the real stream begins.
  - L2 accumulates both batch halves into ONE [64,1024] psum tile
    (different banks) and r1 is a single fused [64,1024] ACT: saves one
    352-cycle ACT instruction start per node on the critical engine.
  - Drain step: half the last node's window reduces run on ACT
    (abs+accum_out) since ACT is idle there; final combine subtract on
    gpsimd; output DMA split in halves (ship cols 0:32 early).

HW model (measured on this kernel's trace):
  ACT ACTIVATE dur ~ 304 + FD ns; DVE TENSOR_REDUCE dur ~ 307 + 1.04*FD
  (PSUM fp32 src is locked to 1 elem/cycle on both engines; GPSIMD has
  no PSUM port). Per node: ACT = 4 h1-chunk relus [128,1024] + fused r1
  = ~6.6us, DVE = 8 window-reduces [128,~550] = ~7.2us, PE (warm,
  2.4GHz) = L1 ~1.0 + L2 ~2.1 + L3 ~3.2 = ~6.3us. The slot is EW-bound
  at ~7us; everything else (DMA, PE clock, fill/drain) must stay off
  the critical path. Framework overhead is ~7.3us preamble (engine
  TENSOR_LOADs + barriers before the first DMA doorbell) + ~9.5us
  postamble (semaphore teardown) and is not controllable from here.

Set CTP_KERNEL_TRACE=1 to capture a neuron-profile trace and print
"HW exec time: <ns> ns".
"""
import sys

sys.path.insert(0, "/opt/trn_rl_repo")

import numpy as np
import ml_dtypes

N_NODES = 64
H = 512
B = 1024
M = 64
N_CORES = 8
JN = 8           # nodes (slots) per core
BC = 8           # batch chunks of 128
BF16 = ml_dtypes.bfloat16

_PROGRAM_CACHE = {}


def _prep(x, W1, W2, W3, b3, W4, b4):
    """Build per-core input maps + program-shape metadata."""
    x = np.asarray(x, np.float32)
    W1 = np.asarray(W1, np.float32)
    W2 = np.asarray(W2, np.float32)
    W3 = np.asarray(W3, np.float32)
    b3 = np.asarray(b3, np.float32)
    W4 = np.asarray(W4, np.float32)
    b4 = np.asarray(b4, np.float32)

    ppos = (W4 >= 0).sum(axis=1)            # sign-split point per node
    order = np.argsort(ppos, kind="stable")
    assign = order.reshape(JN, N_CORES)     # assign[J, core] -> node id
    pmax = np.array([int(ppos[assign[J]].max()) for J in range(JN)])
    pmin = np.array([int(ppos[assign[J]].min()) for J in range(JN)])
    assert pmax.max() < 512, "degenerate all-positive W4 row not supported"
    # symmetric reduce windows: [0:w1) abs+, [w1:2*w1) abs- (zero padded).
    w1 = np.maximum(1 + pmax, 513 - pmin)
    t = 2 * w1                              # per-slot moving width
    # Slot order is arbitrary: run the widest-t group first (its extra
    # reduce width overlaps pipeline fill) and the narrowest last (the
    # drain step's 8 serial reduces scale with t of the final slot).
    perm = np.argsort(-t, kind="stable")
    assign = assign[perm]
    w1 = w1[perm]
    t = t[perm]
    pmax = pmax[perm]
    pmin = pmin[perm]
    tmax = int(t.max())

    xT1 = np.ascontiguousarray(x.T).astype(BF16)         # (64, 1024)
    xT = np.concatenate([xT1, xT1], axis=0)              # (128, 1024) doubled
    in_maps = []
    for j in range(N_CORES):
        g1 = np.zeros((JN, 128, 256), BF16)
        w2 = np.zeros((JN, 128, 4 * M), BF16)
        w3 = np.zeros((JN, 67, tmax), BF16)
        xr = np.zeros((JN, 3, B), BF16)
        for J in range(JN):
            n = int(assign[J, j])
            P = int(ppos[n])
            g1t = W1[n].T.copy()                          # (64 i, 512 h)
            g1t[n, :] = 0.0
            for pair in range(2):
                g1[J][0:64, pair * 128 : (pair + 1) * 128] = \
                    g1t[:, (2 * pair) * 128 : (2 * pair + 1) * 128].astype(BF16)
                g1[J][64:128, pair * 128 : (pair + 1) * 128] = \
                    g1t[:, (2 * pair + 1) * 128 : (2 * pair + 2) * 128].astype(BF16)
            w2t = W2[n].T                                 # (512 h, 64 m)
            w2[J] = np.ascontiguousarray(
                w2t.reshape(4, 128, M).transpose(1, 0, 2).reshape(128, 4 * M)
            ).astype(BF16)

            w4 = W4[n]
            w3ext = np.zeros((66, H), np.float32)
            w3ext[:64] = W3[n, :, :64].T
            w3ext[64] = W3[n, :, 64 + n]
            w3ext[65] = b3[n]
            scaled = w3ext * np.abs(w4)[None, :]
            pos = np.where(w4 >= 0)[0]
            neg = np.where(w4 < 0)[0]
            nmid = pmax[J] - P
            midc, certain = neg[:nmid], neg[nmid:]

            def lift(c):                                  # (66,k) -> (67,k)
                o = np.zeros((67, c.shape[1]), np.float32)
                o[:64] = c[:64]
                o[64] = c[64]
                o[65] = -c[64]
                o[66] = c[65]
                return o

            v = (w3ext @ w4).astype(np.float32)
            v[65] += 2.0 * b4[n]
            v67 = np.zeros(67, np.float32)
            v67[:64] = v[:64]
            v67[64] = v[64]
            v67[65] = -v[64]
            v67[66] = v[65]

            pad = np.zeros((67, tmax), np.float32)
            pad[:, 0] = np.maximum(v67, 0)                       # colA
            pad[:, 1 : 1 + P] = lift(scaled[:, pos])
            pad[:, 1 + P : 1 + pmax[J]] = lift(scaled[:, midc])
            nc_ = len(certain)
            pad[:, w1[J] : w1[J] + nc_] = lift(scaled[:, certain])
            pad[:, w1[J] + nc_ : w1[J] + nc_ + nmid] = 2.0 * lift(scaled[:, midc])
            pad[:, w1[J] + nc_ + nmid] = np.maximum(-v67, 0)     # colB
            w3[J] = pad.astype(BF16)

            xr[J, 0] = np.maximum(x[:, n], 0).astype(BF16)
            xr[J, 1] = np.maximum(-x[:, n], 0).astype(BF16)
            xr[J, 2] = 1.0
        # partition-major packing: node n at free-axis cols [n*w : (n+1)*w]
        g1p = np.ascontiguousarray(g1.transpose(1, 0, 2).reshape(128, JN * 256))
        w2p = np.ascontiguousarray(w2.transpose(1, 0, 2).reshape(128, JN * 4 * M))
        w3p = np.ascontiguousarray(w3.transpose(1, 0, 2).reshape(67, JN * tmax))
        xrp = np.ascontiguousarray(xr.transpose(1, 0, 2).reshape(3, JN * B))
        # every DMA below reads one of these CONTIGUOUS arrays end-to-end
        # (a strided DRAM read serializes onto one SDMA engine: ~25 GB/s)
        cc = lambda a: np.ascontiguousarray(a)
        in_maps.append({
            "xT": xT,
            "g1s0": cc(g1p[:, :256]),
            "g1b1": cc(g1p[:, 256 : 4 * 256]),
            "g1b2": cc(g1p[:, 4 * 256 :]),
            "w2s0": cc(w2p[:, : 4 * M]),
            "w2b1": cc(w2p[:, 4 * M : 4 * 4 * M]),
            "w2b2": cc(w2p[:, 4 * 4 * M :]),
            "w3s0": cc(w3p[:, :tmax]),
            "w3b1": cc(w3p[:, tmax : 4 * tmax]),
            "w3b2": cc(w3p[:, 4 * tmax :]),
            "xrall": xrp,
        })
    return in_maps, assign, tuple(int(v) for v in w1), tuple(int(v) for v in t), tmax


def _build_program(w1, t, tmax):
    import os as _os
    import concourse.bacc as bacc
    import concourse.mybir as mybir
    import concourse.tile as tile

    fp32 = mybir.dt.float32
    bf16 = mybir.dt.bfloat16
    RELU = mybir.ActivationFunctionType.Relu
    ABS = mybir.ActivationFunctionType.Abs
    ADD = mybir.AluOpType.add
    X = mybir.AxisListType.X

    nc = bacc.Bacc("TRN2", target_bir_lowering=False, debug=False,
                   num_devices=N_CORES)
    xT_d = nc.dram_tensor("xT", [128, B], bf16, kind="ExternalInput")
    g1s0_d = nc.dram_tensor("g1s0", [128, 256], bf16, kind="ExternalInput")
    g1b1_d = nc.dram_tensor("g1b1", [128, 3 * 256], bf16, kind="ExternalInput")
    g1b2_d = nc.dram_tensor("g1b2", [128, 4 * 256], bf16, kind="ExternalInput")
    w2s0_d = nc.dram_tensor("w2s0", [128, 4 * M], bf16, kind="ExternalInput")
    w2b1_d = nc.dram_tensor("w2b1", [128, 3 * 4 * M], bf16, kind="ExternalInput")
    w2b2_d = nc.dram_tensor("w2b2", [128, 4 * 4 * M], bf16, kind="ExternalInput")
    w3s0_d = nc.dram_tensor("w3s0", [67, tmax], bf16, kind="ExternalInput")
    w3b1_d = nc.dram_tensor("w3b1", [67, 3 * tmax], bf16, kind="ExternalInput")
    w3b2_d = nc.dram_tensor("w3b2", [67, 4 * tmax], bf16, kind="ExternalInput")
    xr_d = nc.dram_tensor("xrall", [3, JN * B], bf16, kind="ExternalInput")
    out_d = nc.dram_tensor("out", [B, JN], fp32, kind="ExternalOutput")

    with tile.TileContext(nc) as tc:
        with (
            tc.tile_pool(name="const", bufs=1) as const,
            tc.tile_pool(name="act", bufs=2) as apool,
            tc.tile_pool(name="small", bufs=1) as spool,
            tc.tile_pool(name="psa", bufs=2, space="PSUM") as pspool,
            tc.tile_pool(name="psb", bufs=2, space="PSUM") as ps3pool,
        ):
            # static SBUF tiles
            xT_t = const.tile([128, B], bf16, tag="xT")
            g1all = const.tile([128, JN * 256], bf16, tag="g1all")
            w2all = const.tile([128, JN * 4 * M], bf16, tag="w2all")
            w3all = const.tile([67, JN * tmax], bf16, tag="w3all")
            repall = const.tile([67, JN * B], bf16, tag="repall")
            c2 = spool.tile([128, 128], fp32, tag="c2")

            # ---- weight prefetch: 3 queues, contiguous sources, ordered
            # by first use. gpsimd's SWDGE is free here (descriptor
            # generation happens before any compute contends for SBUF).
            # Queue facts (measured): the sync HWDGE ring delivers only
            # ~8-10 GB/s here no matter the shape; the scalar HWDGE ring
            # does 60-85 GB/s; gpsimd SWDGE 35-130 GB/s. So: sync gets
            # only xT (first, small, needed at ~10us), scalar gets all
            # [128, .] weights, SWDGE gets the odd-shaped w3 [67, .] and
            # xr [3, .] (HWDGE serializes those onto one SDMA engine).
            nc.sync.dma_start(xT_t[:], xT_d.ap())
            nc.scalar.dma_start(g1all[:, 0:256], g1s0_d.ap())
            nc.gpsimd.dma_start(repall[64:67, :], xr_d.ap())
            nc.scalar.dma_start(w2all[:, 0 : 4 * M], w2s0_d.ap())
            nc.gpsimd.dma_start(w3all[:, 0:tmax], w3s0_d.ap())
            nc.scalar.dma_start(g1all[:, 256 : 4 * 256], g1b1_d.ap())
            nc.gpsimd.dma_start(w3all[:, tmax : 4 * tmax], w3b1_d.ap())
            nc.scalar.dma_start(w2all[:, 4 * M : 4 * 4 * M], w2b1_d.ap())
            nc.scalar.dma_start(g1all[:, 4 * 256 :], g1b2_d.ap())
            nc.gpsimd.dma_start(w3all[:, 4 * tmax :], w3b2_d.ap())
            nc.scalar.dma_start(w2all[:, 4 * 4 * M :], w2b2_d.ap())

            h1T_t = {}

            # PE warm-up: dummy matmul burst while the DMAs land. The HAM
            # clock gate opens (K=8/8, 2.4 GHz) after ~3.4us of sustained
            # PE busy; without this the whole first half runs at 1.2 GHz.
            # memset on Vector (idle), psum from the ps3 pool (first real
            # use is node 0's L3, ~2 slots in).
            warm_n = int(_os.environ.get("CTP_WARM", "8"))
            warm_s = spool.tile([64, 768], bf16, tag="warm")
            warm_ps = None
            if warm_n:
                nc.vector.memset(warm_s[:], 0)
                warm_ps = ps3pool.tile([128, 512], fp32, tag="ps3",
                                       name="warm_ps")
                for _ in range(warm_n):
                    nc.tensor.matmul(warm_ps[:], warm_s[:, 0:128],
                                     warm_s[:, 256:768], start=True, stop=True)

            def filler(k):
                # fill-phase PE keep-alive: the pipeline isn't full yet, so
                # the PE would idle waiting on the ACT chain / psum-buffer
                # recycling; any idle window re-throttles the HAM clock
                # gate to 1.2 GHz and the cold slots then take ~9us instead
                # of ~5.7 (it takes until ~t=58us to re-lock 2.4 GHz).
                # Dummy FD=512 matmuls in the FIFO bridge those known gaps.
                for _ in range(k):
                    if warm_ps is not None:
                        nc.tensor.matmul(warm_ps[:], warm_s[:, 0:128],
                                         warm_s[:, 256:768], start=True,
                                         stop=True)

            def emit_l1_unit(n, pair):
                # two K=64 matmuls run concurrently in row groups (0,0) and
                # (64,0): h-chunk 2*pair in array rows 0:64, 2*pair+1 in
                # rows 64:128 (g1/xT are laid out accordingly on the host).
                g1_t = g1all[:, n * 256 : (n + 1) * 256]
                h1T = h1T_t[n]
                psA = pspool.tile([128, B], fp32, tag="ps", name=f"psA_{n}_{pair}")
                psB = pspool.tile([128, B], fp32, tag="ps", name=f"psB_{n}_{pair}")
                for bc2 in range(2):
                    nc.tensor.matmul(
                        psA[:, bc2 * 512 : (bc2 + 1) * 512],
                        g1_t[0:64, pair * 128 : (pair + 1) * 128],
                        xT_t[0:64, bc2 * 512 : (bc2 + 1) * 512],
                        start=True, stop=True, tile_position=(0, 0),
                    )
                    nc.tensor.matmul(
                        psB[:, bc2 * 512 : (bc2 + 1) * 512],
                        g1_t[64:128, pair * 128 : (pair + 1) * 128],
                        xT_t[64:128, bc2 * 512 : (bc2 + 1) * 512],
                        start=True, stop=True, tile_position=(64, 0),
                    )
                nc.scalar.activation(
                    h1T[:, (2 * pair) * B : (2 * pair + 1) * B], psA[:], RELU)
                if pair == 1 and _os.environ.get("CTP_C3S", "0") == "1":
                    # split chunk3's ACT into bc halves: L2's hc3 matmuls
                    # (the slot's chain terminal) can issue after the bc0
                    # half instead of idling ACT ~0.65us until the full
                    # [128,1024] copy completes.
                    c3 = (2 * pair + 1) * B
                    nc.scalar.activation(
                        h1T[:, c3 : c3 + 512], psB[:, 0:512], RELU)
                    nc.scalar.activation(
                        h1T[:, c3 + 512 : c3 + B], psB[:, 512:B], RELU)
                else:
                    nc.scalar.activation(
                        h1T[:, (2 * pair + 1) * B : (2 * pair + 2) * B],
                        psB[:], RELU)

            def emit_l2_mm(n, bc2, ps2, hcs):
                # both batch halves accumulate in ONE [64,1024] tile
                # (cols 0:512 = bc half 0 in bank k, 512:1024 = half 1 in
                # bank k+1) so r1 is a single fused ACT.
                w2_t = w2all[:, n * 4 * M : (n + 1) * 4 * M]
                h1T = h1T_t[n]
                for hc in hcs:
                    nc.tensor.matmul(
                        ps2[:, bc2 * 512 : (bc2 + 1) * 512],
                        w2_t[:, hc * M : (hc + 1) * M],
                        h1T[:, hc * B + bc2 * 512 : hc * B + (bc2 + 1) * 512],
                        start=(hc == 0), stop=(hc == 3),
                    )

            def emit_l2_act(n, ps2):
                nc.scalar.activation(
                    repall[0:64, n * B : (n + 1) * B], ps2[:], RELU)

            abs_scratch = spool.tile([128, 320], bf16, tag="abs_scr")

            def emit_l3_unit(n, J, bc, on_act=False):
                w3_t = w3all[:, n * tmax : (n + 1) * tmax]
                rep = repall[:, n * B : (n + 1) * B]
                ps3 = ps3pool.tile([128, B], fp32, tag="ps3")
                stat = rep[:, bc * 128 : (bc + 1) * 128]
                nc.tensor.matmul(ps3[:, 0:512], stat, w3_t[:, 0:512],
                                 start=True, stop=True)
                nc.tensor.matmul(ps3[:, 512 : t[J]], stat, w3_t[:, 512 : t[J]],
                                 start=True, stop=True)
                col = bc * 8 + J
                if on_act:
                    # drain-step offload: ACT is idle, DVE is the drain
                    # bottleneck; Abs+accum_out computes the window sums.
                    wj = t[J] // 2
                    for s in range(2):
                        nc.scalar.activation(
                            abs_scratch[:, 0:wj],
                            ps3[:, s * wj : (s + 1) * wj], ABS,
                            accum_out=c2[:, 2 * col + s : 2 * col + s + 1])
                else:
                    nc.vector.tensor_reduce(
                        c2[:, 2 * col : 2 * col + 2],
                        ps3[:, 0 : t[J]].rearrange("p (s w) -> p s w", s=2),
                        axis=X, op=ADD, apply_absolute_value=True)

            fin_split = _os.environ.get("CTP_FIN", "1") == "1"
            gtt = _os.environ.get("CTP_GTT", "1") == "1"

            def emit_final(cols, osb_tag):
                # out[:, j] = relu(0.5*(cpos - cneg)) for osb col range
                lo, hi = cols
                c3 = c2[:, 2 * lo : 2 * hi].rearrange("p (c s) -> p c s", s=2)
                t1 = spool.tile([128, hi - lo], fp32, tag=osb_tag + "t")
                eng = nc.gpsimd if gtt else nc.vector
                eng.tensor_tensor(t1[:], c3[:, :, 0], c3[:, :, 1],
                                  op=mybir.AluOpType.subtract)
                osb = spool.tile([128, hi - lo], fp32, tag=osb_tag)
                nc.scalar.activation(osb[:], t1[:], RELU, scale=0.5)
                # SWDGE: the sync ring would take ~2us for these 16KB
                nc.gpsimd.dma_start(
                    out_d.ap().rearrange("(k p) n -> p k n", p=128)[:, lo // 8 : hi // 8, :],
                    osb[:].rearrange("p (k n) -> p k n", n=JN),
                )

            # 3-stage software pipeline: step k = L1(k) + L2(k-1) + L3(k-2).
            # Every stage consumes data finished a full step earlier, so no
            # matmul waits on a same-step ACT: the once-per-slot ~0.5us PE
            # stall of the 2-stage version re-throttled the HAM clock gate
            # (K=4/8 for 2 of every 3 windows -> PE at 1.2 GHz half the
            # time despite >93% busy).
            dof = _os.environ.get("CTP_DOF", "1") == "1"
            pipe2 = _os.environ.get("CTP_PIPE", "3") == "2"
            n_steps = JN + 1 if pipe2 else JN + 2
            for step in range(n_steps):
                if pipe2:
                    nxt = step if step < JN else None
                    cur = nxt
                    prev = step - 1 if step > 0 else None
                else:
                    nxt = step if step < JN else None
                    cur = step - 1 if 1 <= step <= JN else None
                    prev = step - 2 if step >= 2 else None
                if nxt is not None:
                    h1T_t[nxt] = apool.tile([128, 4 * B], bf16, tag="h1",
                                            name=f"h1T_{nxt}")
                drain_act = nxt is None and cur is None and dof

                def l3(k):
                    if prev is not None:
                        emit_l3_unit(prev, prev, k,
                                     on_act=drain_act and k in (3, 5, 7))
                fill_n = 0
                if step == 0:
                    fill_n = int(_os.environ.get("CTP_FILL0", "7"))
                elif step == 1 and not pipe2:
                    fill_n = int(_os.environ.get("CTP_FILL1", "5"))
                if nxt is not None:
                    emit_l1_unit(nxt, 0)
                filler(fill_n)
                l3(0); l3(1)
                if nxt is not None:
                    emit_l1_unit(nxt, 1)
                filler(fill_n)
                l3(2); l3(3)
                if fin_split and step == n_steps - 1:
                    # drain step: node 7's bc0-3 reduces just emitted, so
                    # osb cols 0:32 are complete; ship that half early.
                    emit_final((0, 32), "osb0")
                ps2 = None
                if cur is not None:
                    ps2 = pspool.tile([64, B], fp32, tag="ps",
                                      name=f"ps2_{step}")
                    emit_l2_mm(cur, 0, ps2, range(4))
                l3(4); l3(5)
                if cur is not None:
                    emit_l2_mm(cur, 1, ps2, range(4))
                    emit_l2_act(cur, ps2)
                filler(fill_n)
                l3(6); l3(7)
                if cur is not None and not pipe2:
                    h1T_t.pop(cur, None)
                if pipe2 and prev is not None:
                    h1T_t.pop(prev, None)

            if fin_split:
                emit_final((32, 64), "osb1")
            else:
                emit_final((0, 64), "osb")
    nc.compile()
    return nc


def _get_program(w1, t, tmax):
    import os
    key = (w1, t, tmax, os.environ.get("CTP_WARM", ""),
           os.environ.get("CTP_FIN", ""), os.environ.get("CTP_DOF", ""),
           os.environ.get("CTP_C3S", ""), os.environ.get("CTP_GTT", ""),
           os.environ.get("CTP_PIPE", ""))
    if key not in _PROGRAM_CACHE:
        _PROGRAM_CACHE[key] = _build_program(w1, t, tmax)
    return _PROGRAM_CACHE[key]


def kernel(x, W1, W2, W3, b3, W4, b4):
    import os
    from concourse.bass_utils import run_bass_kernel_spmd

    in_maps, assign, w1, t, tmax = _prep(x, W1, W2, W3, b3, W4, b4)
    nc = _get_program(w1, t, tmax)

    trace = os.environ.get("CTP_KERNEL_TRACE", "0") == "1"
    kwargs = {}
    if trace:
        import types
        sys.path.insert(0, "/root/.axon_site")
        from trn_agent_boot.trn_boot import _ntff_profile_via_ctypes
        hook = _ntff_profile_via_ctypes("/opt/axon/libaxon_pjrt.so")
        mod = types.ModuleType("antenv.axon_hooks")
        mod.get_axon_ntff_profile_hook = lambda: hook
        mod.set_axon_ntff_profile_hook = lambda h: None
        sys.modules["antenv.axon_hooks"] = mod
        import concourse.bass_utils as bu
        bu.upload_artifacts = lambda tmpdir: f"local:{tmpdir}"
        tdir = os.environ.get("CTP_TRACE_DIR", "/tmp/ctp_trace")
        os.makedirs(tdir, exist_ok=True)
        kwargs = {"trace": True, "tmpdir": tdir}

    res = run_bass_kernel_spmd(nc, in_maps, list(range(N_CORES)), **kwargs)
    if trace:
        print(f"HW exec time: {res.exec_time_ns} ns")

    out = np.zeros((B, N_NODES), np.float32)
    for j in range(N_CORES):
        oj = np.asarray(res.results[j]["out"], np.float32)   # (B, JN)
        for J in range(JN):
            out[:, int(assign[J, j])] = oj[:, J]
    return out


# revision 15
# speedup vs baseline: 1.0170x; 1.0170x over previous
"""Trainium2 Bass kernel for nn_CausalTrajectoryPrediction.

Math (per node n of 64, batch B=1024):
    h1 = relu(x_masked @ W1[n].T)          x_masked = x with col n zeroed
    r1 = relu(h1 @ W2[n].T)
    h3 = relu([r1, x_n] @ W3[n].T + b3[n])
    out[:, n] = relu(h3 @ W4[n] + b4[n])

Restructuring (validated vs the fp32 reference on CPU):
  - The input mask folds into the weights on the host (zero the diagonal
    column of W1[n]); the "own value" path of W3 collapses to one column;
    b3 becomes a ones-row of the layer-3 stationary operand.
  - The last layer is eliminated via w*relu(z) = 0.5*(w*z + w*|z|):
        out[:, n] = 0.5 * relu(c_pos - c_neg)
    where c_pos/c_neg are DVE abs-reduces over two fixed windows of the
    layer-3 PSUM row. Columns are pre-scaled by |W4| and grouped by
    sign(W4) on the host. The linear term a = rep @ (W3ext@W4 + 2*b4*e1)
    is carried by two extra nonnegative columns (v+ in the positive
    window, v- in the negative window; rep is made elementwise >= 0 by
    splitting x_n into x+/x-), so no extra matmul or PSUM tile is needed.
  - All 8 cores share one program, but the sign-split point differs per
    node. Nodes are assigned to program slots sorted by split point; the
    few "middle" columns that are positive on some cores and negative on
    others sit inside the positive window and are duplicated at the tail
    with weight 2 (|z| - 2|z| = -|z|) on cores where they are negative.
  - Sharding: 8 nodes per core (weights are NOT replicated -> 8x less
    HBM traffic), full batch per core. Host gathers (1024, 8) per core.

v3 changes vs v2 (trace-driven, see HW model below):
  - DMA prefetch: every transfer now reads a CONTIGUOUS DRAM tensor
    (v2 sliced wide tensors; the strided reads landed all descriptors on
    ONE of the 16 SDMA engines -> 25 GB/s, weights not resident until
    t=38us, PE stalling + HAM re-throttling to 1.2 GHz mid-kernel).
    Transfers are spread over three queues (sync/scalar HWDGE + gpsimd
    SWDGE - idle before compute starts) and ordered by first use.
  - Warm-up scratch memset moved to the (otherwise idle) Vector engine
    and the warm psum to the ps3 pool, so the dummy-matmul burst starts
    right after the engine preamble (~5us) instead of ~8us, and the PE's
    HAM clock gate (default K=4/8 = 1.2 GHz; K=8/8 = 2.4 GHz after
    ~3.4us of sustained busy) is fully open when the real stream begins.
  - L2 accumulates both batch halves into ONE [64,1024] psum tile
    (different banks) and r1 is a single fused [64,1024] ACT: saves one
    352-cycle ACT instruction start per node on the critical engine.
  - Drain step: half the last node's window reduces run on ACT
    (abs+accum_out) since ACT is idle there; final combine subtract on
    gpsimd; output DMA split in halves (ship cols 0:32 early).

HW model (measured on this kernel's trace):
  ACT ACTIVATE dur ~ 304 + FD ns; DVE TENSOR_REDUCE dur ~ 307 + 1.04*FD
  (PSUM fp32 src is locked to 1 elem/cycle on both engines; GPSIMD has
  no PSUM port). Per node: ACT = 4 h1-chunk relus [128,1024] + fused r1
  = ~6.6us, DVE = 8 window-reduces [128,~550] = ~7.2us, PE (warm,
  2.4GHz) = L1 ~1.0 + L2 ~2.1 + L3 ~3.2 = ~6.3us. The slot is EW-bound
  at ~7us; everything else (DMA, PE clock, fill/drain) must stay off
  the critical path. Framework overhead is ~7.3us preamble (engine
  TENSOR_LOADs + barriers before the first DMA doorbell) + ~9.5us
  postamble (semaphore teardown) and is not controllable from here.

Set CTP_KERNEL_TRACE=1 to capture a neuron-profile trace and print
"HW exec time: <ns> ns".
"""
import sys

sys.path.insert(0, "/opt/trn_rl_repo")

import numpy as np
import ml_dtypes

N_NODES = 64
H = 512
B = 1024
M = 64
N_CORES = 8
JN = 8           # nodes (slots) per core
BC = 8           # batch chunks of 128
BF16 = ml_dtypes.bfloat16

_PROGRAM_CACHE = {}


def _prep(x, W1, W2, W3, b3, W4, b4):
    """Build per-core input maps + program-shape metadata."""
    x = np.asarray(x, np.float32)
    W1 = np.asarray(W1, np.float32)
    W2 = np.asarray(W2, np.float32)
    W3 = np.asarray(W3, np.float32)
    b3 = np.asarray(b3, np.float32)
    W4 = np.asarray(W4, np.float32)
    b4 = np.asarray(b4, np.float32)

    ppos = (W4 >= 0).sum(axis=1)            # sign-split point per node
    order = np.argsort(ppos, kind="stable")
    assign = order.reshape(JN, N_CORES)     # assign[J, core] -> node id
    pmax = np.array([int(ppos[assign[J]].max()) for J in range(JN)])
    pmin = np.array([int(ppos[assign[J]].min()) for J in range(JN)])
    assert pmax.max() < 512, "degenerate all-positive W4 row not supported"
    # symmetric reduce windows: [0:w1) abs+, [w1:2*w1) abs- (zero padded).
    w1 = np.maximum(1 + pmax, 513 - pmin)
    t = 2 * w1                              # per-slot moving width
    # Slot order is arbitrary: run the widest-t group first (its extra
    # reduce width overlaps pipeline fill) and the narrowest last (the
    # drain step's 8 serial reduces scale with t of the final slot).
    perm = np.argsort(-t, kind="stable")
    assign = assign[perm]
    w1 = w1[perm]
    t = t[perm]
    pmax = pmax[perm]
    pmin = pmin[perm]
    tmax = int(t.max())

    xT1 = np.ascontiguousarray(x.T).astype(BF16)         # (64, 1024)
    xT = np.concatenate([xT1, xT1], axis=0)              # (128, 1024) doubled
    in_maps = []
    for j in range(N_CORES):
        g1 = np.zeros((JN, 128, 256), BF16)
        w2 = np.zeros((JN, 128, 4 * M), BF16)
        w3 = np.zeros((JN, 67, tmax), BF16)
        xr = np.zeros((JN, 3, B), BF16)
        for J in range(JN):
            n = int(assign[J, j])
            P = int(ppos[n])
            g1t = W1[n].T.copy()                          # (64 i, 512 h)
            g1t[n, :] = 0.0
            for pair in range(2):
                g1[J][0:64, pair * 128 : (pair + 1) * 128] = \
                    g1t[:, (2 * pair) * 128 : (2 * pair + 1) * 128].astype(BF16)
                g1[J][64:128, pair * 128 : (pair + 1) * 128] = \
                    g1t[:, (2 * pair + 1) * 128 : (2 * pair + 2) * 128].astype(BF16)
            w2t = W2[n].T                                 # (512 h, 64 m)
            w2[J] = np.ascontiguousarray(
                w2t.reshape(4, 128, M).transpose(1, 0, 2).reshape(128, 4 * M)
            ).astype(BF16)

            w4 = W4[n]
            w3ext = np.zeros((66, H), np.float32)
            w3ext[:64] = W3[n, :, :64].T
            w3ext[64] = W3[n, :, 64 + n]
            w3ext[65] = b3[n]
            scaled = w3ext * np.abs(w4)[None, :]
            pos = np.where(w4 >= 0)[0]
            neg = np.where(w4 < 0)[0]
            nmid = pmax[J] - P
            midc, certain = neg[:nmid], neg[nmid:]

            def lift(c):                                  # (66,k) -> (67,k)
                o = np.zeros((67, c.shape[1]), np.float32)
                o[:64] = c[:64]
                o[64] = c[64]
                o[65] = -c[64]
                o[66] = c[65]
                return o

            v = (w3ext @ w4).astype(np.float32)
            v[65] += 2.0 * b4[n]
            v67 = np.zeros(67, np.float32)
            v67[:64] = v[:64]
            v67[64] = v[64]
            v67[65] = -v[64]
            v67[66] = v[65]

            pad = np.zeros((67, tmax), np.float32)
            pad[:, 0] = np.maximum(v67, 0)                       # colA
            pad[:, 1 : 1 + P] = lift(scaled[:, pos])
            pad[:, 1 + P : 1 + pmax[J]] = lift(scaled[:, midc])
            nc_ = len(certain)
            pad[:, w1[J] : w1[J] + nc_] = lift(scaled[:, certain])
            pad[:, w1[J] + nc_ : w1[J] + nc_ + nmid] = 2.0 * lift(scaled[:, midc])
            pad[:, w1[J] + nc_ + nmid] = np.maximum(-v67, 0)     # colB
            w3[J] = pad.astype(BF16)

            xr[J, 0] = np.maximum(x[:, n], 0).astype(BF16)
            xr[J, 1] = np.maximum(-x[:, n], 0).astype(BF16)
            xr[J, 2] = 1.0
        # partition-major packing: node n at free-axis cols [n*w : (n+1)*w]
        g1p = np.ascontiguousarray(g1.transpose(1, 0, 2).reshape(128, JN * 256))
        w2p = np.ascontiguousarray(w2.transpose(1, 0, 2).reshape(128, JN * 4 * M))
        w3p = np.ascontiguousarray(w3.transpose(1, 0, 2).reshape(67, JN * tmax))
        xrp = np.ascontiguousarray(xr.transpose(1, 0, 2).reshape(3, JN * B))
        # every DMA below reads one of these CONTIGUOUS arrays end-to-end
        # (a strided DRAM read serializes onto one SDMA engine: ~25 GB/s)
        cc = lambda a: np.ascontiguousarray(a)
        in_maps.append({
            "xT": xT,
            "g1s0": cc(g1p[:, :256]),
            "g1b1": cc(g1p[:, 256 : 4 * 256]),
            "g1b2": cc(g1p[:, 4 * 256 :]),
            "w2s0": cc(w2p[:, : 4 * M]),
            "w2b1": cc(w2p[:, 4 * M : 4 * 4 * M]),
            "w2b2": cc(w2p[:, 4 * 4 * M :]),
            "w3s0": cc(w3p[:, :tmax]),
            "w3b1": cc(w3p[:, tmax : 4 * tmax]),
            "w3b2": cc(w3p[:, 4 * tmax :]),
            "xrall": xrp,
        })
    return in_maps, assign, tuple(int(v) for v in w1), tuple(int(v) for v in t), tmax


def _build_program(w1, t, tmax):
    import os as _os
    import concourse.bacc as bacc
    import concourse.mybir as mybir
    import concourse.tile as tile

    fp32 = mybir.dt.float32
    bf16 = mybir.dt.bfloat16
    RELU = mybir.ActivationFunctionType.Relu
    ABS = mybir.ActivationFunctionType.Abs
    ADD = mybir.AluOpType.add
    X = mybir.AxisListType.X

    nc = bacc.Bacc("TRN2", target_bir_lowering=False, debug=False,
                   num_devices=N_CORES)
    xT_d = nc.dram_tensor("xT", [128, B], bf16, kind="ExternalInput")
    g1s0_d = nc.dram_tensor("g1s0", [128, 256], bf16, kind="ExternalInput")
    g1b1_d = nc.dram_tensor("g1b1", [128, 3 * 256], bf16, kind="ExternalInput")
    g1b2_d = nc.dram_tensor("g1b2", [128, 4 * 256], bf16, kind="ExternalInput")
    w2s0_d = nc.dram_tensor("w2s0", [128, 4 * M], bf16, kind="ExternalInput")
    w2b1_d = nc.dram_tensor("w2b1", [128, 3 * 4 * M], bf16, kind="ExternalInput")
    w2b2_d = nc.dram_tensor("w2b2", [128, 4 * 4 * M], bf16, kind="ExternalInput")
    w3s0_d = nc.dram_tensor("w3s0", [67, tmax], bf16, kind="ExternalInput")
    w3b1_d = nc.dram_tensor("w3b1", [67, 3 * tmax], bf16, kind="ExternalInput")
    w3b2_d = nc.dram_tensor("w3b2", [67, 4 * tmax], bf16, kind="ExternalInput")
    xr_d = nc.dram_tensor("xrall", [3, JN * B], bf16, kind="ExternalInput")
    out_d = nc.dram_tensor("out", [B, JN], fp32, kind="ExternalOutput")

    with tile.TileContext(nc) as tc:
        with (
            tc.tile_pool(name="const", bufs=1) as const,
            tc.tile_pool(name="act", bufs=2) as apool,
            tc.tile_pool(name="small", bufs=1) as spool,
            tc.tile_pool(name="psa", bufs=2, space="PSUM") as pspool,
            tc.tile_pool(name="psb", bufs=2, space="PSUM") as ps3pool,
        ):
            # static SBUF tiles
            xT_t = const.tile([128, B], bf16, tag="xT")
            g1all = const.tile([128, JN * 256], bf16, tag="g1all")
            w2all = const.tile([128, JN * 4 * M], bf16, tag="w2all")
            w3all = const.tile([67, JN * tmax], bf16, tag="w3all")
            repall = const.tile([67, JN * B], bf16, tag="repall")
            c2 = spool.tile([128, 128], fp32, tag="c2")

            # ---- weight prefetch: 3 queues, contiguous sources, ordered
            # by first use. gpsimd's SWDGE is free here (descriptor
            # generation happens before any compute contends for SBUF).
            # Queue facts (measured): the sync HWDGE ring delivers only
            # ~8-10 GB/s here no matter the shape; the scalar HWDGE ring
            # does 60-85 GB/s; gpsimd SWDGE 35-130 GB/s. So: sync gets
            # only xT (first, small, needed at ~10us), scalar gets all
            # [128, .] weights, SWDGE gets the odd-shaped w3 [67, .] and
            # xr [3, .] (HWDGE serializes those onto one SDMA engine).
            nc.sync.dma_start(xT_t[:], xT_d.ap())
            nc.scalar.dma_start(g1all[:, 0:256], g1s0_d.ap())
            nc.gpsimd.dma_start(repall[64:67, :], xr_d.ap())
            nc.scalar.dma_start(w2all[:, 0 : 4 * M], w2s0_d.ap())
            nc.gpsimd.dma_start(w3all[:, 0:tmax], w3s0_d.ap())
            nc.scalar.dma_start(g1all[:, 256 : 4 * 256], g1b1_d.ap())
            nc.gpsimd.dma_start(w3all[:, tmax : 4 * tmax], w3b1_d.ap())
            nc.scalar.dma_start(w2all[:, 4 * M : 4 * 4 * M], w2b1_d.ap())
            nc.scalar.dma_start(g1all[:, 4 * 256 :], g1b2_d.ap())
            nc.gpsimd.dma_start(w3all[:, 4 * tmax :], w3b2_d.ap())
            nc.scalar.dma_start(w2all[:, 4 * 4 * M :], w2b2_d.ap())

            h1T_t = {}

            # PE warm-up: dummy matmul burst while the DMAs land. The HAM
            # clock gate opens (K=8/8, 2.4 GHz) after ~3.4us of sustained
            # PE busy; without this the whole first half runs at 1.2 GHz.
            # memset on Vector (idle), psum from the ps3 pool (first real
            # use is node 0's L3, ~2 slots in).
            warm_n = int(_os.environ.get("CTP_WARM", "8"))
            warm_s = spool.tile([64, 768], bf16, tag="warm")
            warm_ps = None
            if warm_n:
                nc.vector.memset(warm_s[:], 0)
                warm_ps = ps3pool.tile([128, 512], fp32, tag="ps3",
                                       name="warm_ps")
                for _ in range(warm_n):
                    nc.tensor.matmul(warm_ps[:], warm_s[:, 0:128],
                                     warm_s[:, 256:768], start=True, stop=True)

            def filler(k):
                # fill-phase PE keep-alive: the pipeline isn't full yet, so
                # the PE would idle waiting on the ACT chain / psum-buffer
                # recycling; any idle window re-throttles the HAM clock
                # gate to 1.2 GHz and the cold slots then take ~9us instead
                # of ~5.7 (it takes until ~t=58us to re-lock 2.4 GHz).
                # Dummy FD=512 matmuls in the FIFO bridge those known gaps.
                # FD=128: ~57ns warm / ~107ns cold each, so a misjudged
                # count displaces little real work.
                for _ in range(k):
                    if warm_ps is not None:
                        nc.tensor.matmul(warm_ps[:, 0:128], warm_s[:, 0:128],
                                         warm_s[:, 256:384], start=True,
                                         stop=True)

            def emit_l1_unit(n, pair):
                # two K=64 matmuls run concurrently in row groups (0,0) and
                # (64,0): h-chunk 2*pair in array rows 0:64, 2*pair+1 in
                # rows 64:128 (g1/xT are laid out accordingly on the host).
                g1_t = g1all[:, n * 256 : (n + 1) * 256]
                h1T = h1T_t[n]
                psA = pspool.tile([128, B], fp32, tag="ps", name=f"psA_{n}_{pair}")
                psB = pspool.tile([128, B], fp32, tag="ps", name=f"psB_{n}_{pair}")
                for bc2 in range(2):
                    nc.tensor.matmul(
                        psA[:, bc2 * 512 : (bc2 + 1) * 512],
                        g1_t[0:64, pair * 128 : (pair + 1) * 128],
                        xT_t[0:64, bc2 * 512 : (bc2 + 1) * 512],
                        start=True, stop=True, tile_position=(0, 0),
                    )
                    nc.tensor.matmul(
                        psB[:, bc2 * 512 : (bc2 + 1) * 512],
                        g1_t[64:128, pair * 128 : (pair + 1) * 128],
                        xT_t[64:128, bc2 * 512 : (bc2 + 1) * 512],
                        start=True, stop=True, tile_position=(64, 0),
                    )
                nc.scalar.activation(
                    h1T[:, (2 * pair) * B : (2 * pair + 1) * B], psA[:], RELU)
                if pair == 1 and _os.environ.get("CTP_C3S", "0") == "1":
                    # split chunk3's ACT into bc halves: L2's hc3 matmuls
                    # (the slot's chain terminal) can issue after the bc0
                    # half instead of idling ACT ~0.65us until the full
                    # [128,1024] copy completes.
                    c3 = (2 * pair + 1) * B
                    nc.scalar.activation(
                        h1T[:, c3 : c3 + 512], psB[:, 0:512], RELU)
                    nc.scalar.activation(
                        h1T[:, c3 + 512 : c3 + B], psB[:, 512:B], RELU)
                else:
                    nc.scalar.activation(
                        h1T[:, (2 * pair + 1) * B : (2 * pair + 2) * B],
                        psB[:], RELU)

            colt = _os.environ.get("CTP_COLT", "0") == "1"

            def emit_l2_mm(n, bc2, ps2, hcs):
                # default: both batch halves accumulate in ONE [64,1024]
                # tile (cols 0:512 = bc half 0 in bank k, 512:1024 = half
                # 1 in bank k+1) so r1 is a single fused ACT.
                # CTP_COLT=1: half 1 instead goes to PE column group
                # (0,64) -> psum partitions 64:128, same columns; the two
                # 4-chunk accumulations then stream CONCURRENTLY through
                # the PE (measured 2.38x for col-tiling) halving L2's PE
                # time, at the cost of two r1 ACTs (one partition-shifted).
                w2_t = w2all[:, n * 4 * M : (n + 1) * 4 * M]
                h1T = h1T_t[n]
                for hc in hcs:
                    if colt and bc2 == 1:
                        out = ps2[64:128, 0:512]
                        kw = {"tile_position": (0, 64)}
                    else:
                        out = ps2[0:64, bc2 * 512 : (bc2 + 1) * 512]
                        kw = {"tile_position": (0, 0)} if colt else {}
                    nc.tensor.matmul(
                        out,
                        w2_t[:, hc * M : (hc + 1) * M],
                        h1T[:, hc * B + bc2 * 512 : hc * B + (bc2 + 1) * 512],
                        start=(hc == 0), stop=(hc == 3), **kw,
                    )

            def emit_l2_act(n, ps2):
                if colt:
                    nc.scalar.activation(
                        repall[0:64, n * B : n * B + 512],
                        ps2[0:64, 0:512], RELU)
                    nc.scalar.activation(
                        repall[0:64, n * B + 512 : (n + 1) * B],
                        ps2[64:128, 0:512], RELU)
                else:
                    nc.scalar.activation(
                        repall[0:64, n * B : (n + 1) * B], ps2[0:64, :], RELU)

            abs_scratch = spool.tile([128, 320], bf16, tag="abs_scr")

            def emit_l3_unit(n, J, bc, on_act=False):
                w3_t = w3all[:, n * tmax : (n + 1) * tmax]
                rep = repall[:, n * B : (n + 1) * B]
                ps3 = ps3pool.tile([128, B], fp32, tag="ps3")
                stat = rep[:, bc * 128 : (bc + 1) * 128]
                nc.tensor.matmul(ps3[:, 0:512], stat, w3_t[:, 0:512],
                                 start=True, stop=True)
                nc.tensor.matmul(ps3[:, 512 : t[J]], stat, w3_t[:, 512 : t[J]],
                                 start=True, stop=True)
                col = bc * 8 + J
                if on_act:
                    # drain-step offload: ACT is idle, DVE is the drain
                    # bottleneck; Abs+accum_out computes the window sums.
                    wj = t[J] // 2
                    for s in range(2):
                        nc.scalar.activation(
                            abs_scratch[:, 0:wj],
                            ps3[:, s * wj : (s + 1) * wj], ABS,
                            accum_out=c2[:, 2 * col + s : 2 * col + s + 1])
                else:
                    nc.vector.tensor_reduce(
                        c2[:, 2 * col : 2 * col + 2],
                        ps3[:, 0 : t[J]].rearrange("p (s w) -> p s w", s=2),
                        axis=X, op=ADD, apply_absolute_value=True)

            fin_split = _os.environ.get("CTP_FIN", "1") == "1"
            gtt = _os.environ.get("CTP_GTT", "1") == "1"

            def emit_final(cols, osb_tag):
                # out[:, j] = relu(0.5*(cpos - cneg)) for osb col range
                lo, hi = cols
                c3 = c2[:, 2 * lo : 2 * hi].rearrange("p (c s) -> p c s", s=2)
                t1 = spool.tile([128, hi - lo], fp32, tag=osb_tag + "t")
                eng = nc.gpsimd if gtt else nc.vector
                eng.tensor_tensor(t1[:], c3[:, :, 0], c3[:, :, 1],
                                  op=mybir.AluOpType.subtract)
                osb = spool.tile([128, hi - lo], fp32, tag=osb_tag)
                nc.scalar.activation(osb[:], t1[:], RELU, scale=0.5)
                # SWDGE: the sync ring would take ~2us for these 16KB
                nc.gpsimd.dma_start(
                    out_d.ap().rearrange("(k p) n -> p k n", p=128)[:, lo // 8 : hi // 8, :],
                    osb[:].rearrange("p (k n) -> p k n", n=JN),
                )

            # 3-stage software pipeline: step k = L1(k) + L2(k-1) + L3(k-2).
            # Every stage consumes data finished a full step earlier, so no
            # matmul waits on a same-step ACT: the once-per-slot ~0.5us PE
            # stall of the 2-stage version re-throttled the HAM clock gate
            # (K=4/8 for 2 of every 3 windows -> PE at 1.2 GHz half the
            # time despite >93% busy).
            dof = _os.environ.get("CTP_DOF", "1") == "1"
            pipe2 = _os.environ.get("CTP_PIPE", "3") == "2"
            n_steps = JN + 1 if pipe2 else JN + 2
            for step in range(n_steps):
                if pipe2:
                    nxt = step if step < JN else None
                    cur = nxt
                    prev = step - 1 if step > 0 else None
                else:
                    nxt = step if step < JN else None
                    cur = step - 1 if 1 <= step <= JN else None
                    prev = step - 2 if step >= 2 else None
                if nxt is not None:
                    h1T_t[nxt] = apool.tile([128, 4 * B], bf16, tag="h1",
                                            name=f"h1T_{nxt}")
                drain_act = nxt is None and cur is None and dof

                def l3(k):
                    if prev is not None:
                        emit_l3_unit(prev, prev, k,
                                     on_act=drain_act and k in (3, 5, 7))
                fill_n = 0
                if step == 0:
                    fill_n = int(_os.environ.get("CTP_FILL0", "7"))
                elif step == 1 and not pipe2:
                    fill_n = int(_os.environ.get("CTP_FILL1", "5"))
                if nxt is not None:
                    emit_l1_unit(nxt, 0)
                filler(fill_n)
                l3(0); l3(1)
                if nxt is not None:
                    emit_l1_unit(nxt, 1)
                filler(fill_n)
                l3(2); l3(3)
                if fin_split and step == n_steps - 1:
                    # drain step: node 7's bc0-3 reduces just emitted, so
                    # osb cols 0:32 are complete; ship that half early.
                    emit_final((0, 32), "osb0")
                ps2 = None
                if cur is not None:
                    if colt:
                        ps2 = pspool.tile([128, 512], fp32, tag="ps",
                                          name=f"ps2_{step}")
                        # interleave the halves' chunks so the two column
                        # groups stream through the PE concurrently
                        for hc in range(2):
                            emit_l2_mm(cur, 0, ps2, [hc])
                            emit_l2_mm(cur, 1, ps2, [hc])
                    else:
                        ps2 = pspool.tile([64, B], fp32, tag="ps",
                                          name=f"ps2_{step}")
                        emit_l2_mm(cur, 0, ps2, range(4))
                l3(4); l3(5)
                if cur is not None:
                    if colt:
                        for hc in range(2, 4):
                            emit_l2_mm(cur, 0, ps2, [hc])
                            emit_l2_mm(cur, 1, ps2, [hc])
                    else:
                        emit_l2_mm(cur, 1, ps2, range(4))
                    emit_l2_act(cur, ps2)
                filler(fill_n)
                l3(6); l3(7)
                if cur is not None and not pipe2:
                    h1T_t.pop(cur, None)
                if pipe2 and prev is not None:
                    h1T_t.pop(prev, None)

            if fin_split:
                emit_final((32, 64), "osb1")
            else:
                emit_final((0, 64), "osb")
    nc.compile()
    return nc


def _get_program(w1, t, tmax):
    import os
    key = (w1, t, tmax, os.environ.get("CTP_WARM", ""),
           os.environ.get("CTP_FIN", ""), os.environ.get("CTP_DOF", ""),
           os.environ.get("CTP_C3S", ""), os.environ.get("CTP_GTT", ""),
           os.environ.get("CTP_PIPE", ""), os.environ.get("CTP_COLT", ""),
           os.environ.get("CTP_FILL0", ""), os.environ.get("CTP_FILL1", ""))
    if key not in _PROGRAM_CACHE:
        _PROGRAM_CACHE[key] = _build_program(w1, t, tmax)
    return _PROGRAM_CACHE[key]


def kernel(x, W1, W2, W3, b3, W4, b4):
    import os
    from concourse.bass_utils import run_bass_kernel_spmd

    in_maps, assign, w1, t, tmax = _prep(x, W1, W2, W3, b3, W4, b4)
    nc = _get_program(w1, t, tmax)

    trace = os.environ.get("CTP_KERNEL_TRACE", "0") == "1"
    kwargs = {}
    if trace:
        import types
        sys.path.insert(0, "/root/.axon_site")
        from trn_agent_boot.trn_boot import _ntff_profile_via_ctypes
        hook = _ntff_profile_via_ctypes("/opt/axon/libaxon_pjrt.so")
        mod = types.ModuleType("antenv.axon_hooks")
        mod.get_axon_ntff_profile_hook = lambda: hook
        mod.set_axon_ntff_profile_hook = lambda h: None
        sys.modules["antenv.axon_hooks"] = mod
        import concourse.bass_utils as bu
        bu.upload_artifacts = lambda tmpdir: f"local:{tmpdir}"
        tdir = os.environ.get("CTP_TRACE_DIR", "/tmp/ctp_trace")
        os.makedirs(tdir, exist_ok=True)
        kwargs = {"trace": True, "tmpdir": tdir}

    res = run_bass_kernel_spmd(nc, in_maps, list(range(N_CORES)), **kwargs)
    if trace:
        print(f"HW exec time: {res.exec_time_ns} ns")

    out = np.zeros((B, N_NODES), np.float32)
    for j in range(N_CORES):
        oj = np.asarray(res.results[j]["out"], np.float32)   # (B, JN)
        for J in range(JN):
            out[:, int(assign[J, j])] = oj[:, J]
    return out


# revision 18
# speedup vs baseline: 1.0833x; 1.0653x over previous
"""Trainium2 Bass kernel for nn_CausalTrajectoryPrediction.

Math (per node n of 64, batch B=1024):
    h1 = relu(x_masked @ W1[n].T)          x_masked = x with col n zeroed
    r1 = relu(h1 @ W2[n].T)
    h3 = relu([r1, x_n] @ W3[n].T + b3[n])
    out[:, n] = relu(h3 @ W4[n] + b4[n])

Restructuring (validated vs the fp32 reference on CPU):
  - The input mask folds into the weights on the host (zero the diagonal
    column of W1[n]); the "own value" path of W3 collapses to one column;
    b3 becomes a ones-row of the layer-3 stationary operand.
  - The last layer is eliminated via w*relu(z) = 0.5*(w*z + w*|z|):
        out[:, n] = 0.5 * relu(c_pos - c_neg)
    where c_pos/c_neg are DVE abs-reduces over two fixed windows of the
    layer-3 PSUM row. Columns are pre-scaled by |W4| and grouped by
    sign(W4) on the host. The linear term a = rep @ (W3ext@W4 + 2*b4*e1)
    is carried by two extra nonnegative columns (v+ in the positive
    window, v- in the negative window; rep is made elementwise >= 0 by
    splitting x_n into x+/x-), so no extra matmul or PSUM tile is needed.
  - All 8 cores share one program, but the sign-split point differs per
    node. Nodes are assigned to program slots sorted by split point; the
    few "middle" columns that are positive on some cores and negative on
    others sit inside the positive window and are duplicated at the tail
    with weight 2 (|z| - 2|z| = -|z|) on cores where they are negative.
  - Sharding: 8 nodes per core (weights are NOT replicated -> 8x less
    HBM traffic), full batch per core. Host gathers (1024, 8) per core.

v3 changes vs v2 (trace-driven, see HW model below):
  - DMA prefetch: every transfer now reads a CONTIGUOUS DRAM tensor
    (v2 sliced wide tensors; the strided reads landed all descriptors on
    ONE of the 16 SDMA engines -> 25 GB/s, weights not resident until
    t=38us, PE stalling + HAM re-throttling to 1.2 GHz mid-kernel).
    Transfers are spread over three queues (sync/scalar HWDGE + gpsimd
    SWDGE - idle before compute starts) and ordered by first use.
  - Warm-up scratch memset moved to the (otherwise idle) Vector engine
    and the warm psum to the ps3 pool, so the dummy-matmul burst starts
    right after the engine preamble (~5us) instead of ~8us, and the PE's
    HAM clock gate (default K=4/8 = 1.2 GHz; K=8/8 = 2.4 GHz after
    ~3.4us of sustained busy) is fully open when the real stream begins.
  - L2 accumulates both batch halves into ONE [64,1024] psum tile
    (different banks) and r1 is a single fused [64,1024] ACT: saves one
    352-cycle ACT instruction start per node on the critical engine.
  - Drain step: half the last node's window reduces run on ACT
    (abs+accum_out) since ACT is idle there; final combine subtract on
    gpsimd; output DMA split in halves (ship cols 0:32 early).

HW model (measured on this kernel's trace):
  ACT ACTIVATE dur ~ 304 + FD ns; DVE TENSOR_REDUCE dur ~ 307 + 1.04*FD
  (PSUM fp32 src is locked to 1 elem/cycle on both engines; GPSIMD has
  no PSUM port). Per node: ACT = 4 h1-chunk relus [128,1024] + fused r1
  = ~6.6us, DVE = 8 window-reduces [128,~550] = ~7.2us, PE (warm,
  2.4GHz) = L1 ~1.0 + L2 ~2.1 + L3 ~3.2 = ~6.3us. The slot is EW-bound
  at ~7us; everything else (DMA, PE clock, fill/drain) must stay off
  the critical path. Framework overhead is ~7.3us preamble (engine
  TENSOR_LOADs + barriers before the first DMA doorbell) + ~9.5us
  postamble (semaphore teardown) and is not controllable from here.

Set CTP_KERNEL_TRACE=1 to capture a neuron-profile trace and print
"HW exec time: <ns> ns".
"""
import sys

sys.path.insert(0, "/opt/trn_rl_repo")

import numpy as np
import ml_dtypes

N_NODES = 64
H = 512
B = 1024
M = 64
N_CORES = 8
JN = 8           # nodes (slots) per core
BC = 8           # batch chunks of 128
BF16 = ml_dtypes.bfloat16

_PROGRAM_CACHE = {}


def _prep(x, W1, W2, W3, b3, W4, b4):
    """Build per-core input maps + program-shape metadata."""
    x = np.asarray(x, np.float32)
    W1 = np.asarray(W1, np.float32)
    W2 = np.asarray(W2, np.float32)
    W3 = np.asarray(W3, np.float32)
    b3 = np.asarray(b3, np.float32)
    W4 = np.asarray(W4, np.float32)
    b4 = np.asarray(b4, np.float32)

    ppos = (W4 >= 0).sum(axis=1)            # sign-split point per node
    order = np.argsort(ppos, kind="stable")
    assign = order.reshape(JN, N_CORES)     # assign[J, core] -> node id
    pmax = np.array([int(ppos[assign[J]].max()) for J in range(JN)])
    pmin = np.array([int(ppos[assign[J]].min()) for J in range(JN)])
    assert pmax.max() < 512, "degenerate all-positive W4 row not supported"
    # symmetric reduce windows: [0:w1) abs+, [w1:2*w1) abs- (zero padded).
    w1 = np.maximum(1 + pmax, 513 - pmin)
    t = 2 * w1                              # per-slot moving width
    # Slot order is arbitrary: run the widest-t group first (its extra
    # reduce width overlaps pipeline fill) and the narrowest last (the
    # drain step's 8 serial reduces scale with t of the final slot).
    perm = np.argsort(-t, kind="stable")
    assign = assign[perm]
    w1 = w1[perm]
    t = t[perm]
    pmax = pmax[perm]
    pmin = pmin[perm]
    tmax = int(t.max())

    xT1 = np.ascontiguousarray(x.T).astype(BF16)         # (64, 1024)
    xT = np.concatenate([xT1, xT1], axis=0)              # (128, 1024) doubled
    in_maps = []
    for j in range(N_CORES):
        g1 = np.zeros((JN, 128, 256), BF16)
        w2 = np.zeros((JN, 128, 4 * M), BF16)
        w3 = np.zeros((JN, 67, tmax), BF16)
        xr = np.zeros((JN, 3, B), BF16)
        for J in range(JN):
            n = int(assign[J, j])
            P = int(ppos[n])
            g1t = W1[n].T.copy()                          # (64 i, 512 h)
            g1t[n, :] = 0.0
            for pair in range(2):
                g1[J][0:64, pair * 128 : (pair + 1) * 128] = \
                    g1t[:, (2 * pair) * 128 : (2 * pair + 1) * 128].astype(BF16)
                g1[J][64:128, pair * 128 : (pair + 1) * 128] = \
                    g1t[:, (2 * pair + 1) * 128 : (2 * pair + 2) * 128].astype(BF16)
            w2t = W2[n].T                                 # (512 h, 64 m)
            w2[J] = np.ascontiguousarray(
                w2t.reshape(4, 128, M).transpose(1, 0, 2).reshape(128, 4 * M)
            ).astype(BF16)

            w4 = W4[n]
            w3ext = np.zeros((66, H), np.float32)
            w3ext[:64] = W3[n, :, :64].T
            w3ext[64] = W3[n, :, 64 + n]
            w3ext[65] = b3[n]
            scaled = w3ext * np.abs(w4)[None, :]
            pos = np.where(w4 >= 0)[0]
            neg = np.where(w4 < 0)[0]
            nmid = pmax[J] - P
            midc, certain = neg[:nmid], neg[nmid:]

            def lift(c):                                  # (66,k) -> (67,k)
                o = np.zeros((67, c.shape[1]), np.float32)
                o[:64] = c[:64]
                o[64] = c[64]
                o[65] = -c[64]
                o[66] = c[65]
                return o

            v = (w3ext @ w4).astype(np.float32)
            v[65] += 2.0 * b4[n]
            v67 = np.zeros(67, np.float32)
            v67[:64] = v[:64]
            v67[64] = v[64]
            v67[65] = -v[64]
            v67[66] = v[65]

            pad = np.zeros((67, tmax), np.float32)
            pad[:, 0] = np.maximum(v67, 0)                       # colA
            pad[:, 1 : 1 + P] = lift(scaled[:, pos])
            pad[:, 1 + P : 1 + pmax[J]] = lift(scaled[:, midc])
            nc_ = len(certain)
            pad[:, w1[J] : w1[J] + nc_] = lift(scaled[:, certain])
            pad[:, w1[J] + nc_ : w1[J] + nc_ + nmid] = 2.0 * lift(scaled[:, midc])
            pad[:, w1[J] + nc_ + nmid] = np.maximum(-v67, 0)     # colB
            w3[J] = pad.astype(BF16)

            xr[J, 0] = np.maximum(x[:, n], 0).astype(BF16)
            xr[J, 1] = np.maximum(-x[:, n], 0).astype(BF16)
            xr[J, 2] = 1.0
        # partition-major packing: node n at free-axis cols [n*w : (n+1)*w]
        g1p = np.ascontiguousarray(g1.transpose(1, 0, 2).reshape(128, JN * 256))
        w2p = np.ascontiguousarray(w2.transpose(1, 0, 2).reshape(128, JN * 4 * M))
        w3p = np.ascontiguousarray(w3.transpose(1, 0, 2).reshape(67, JN * tmax))
        xrp = np.ascontiguousarray(xr.transpose(1, 0, 2).reshape(3, JN * B))
        # every DMA below reads one of these CONTIGUOUS arrays end-to-end
        # (a strided DRAM read serializes onto one SDMA engine: ~25 GB/s)
        cc = lambda a: np.ascontiguousarray(a)
        in_maps.append({
            "xT": xT,
            "g1s0": cc(g1p[:, :256]),
            "g1b1": cc(g1p[:, 256 : 4 * 256]),
            "g1b2": cc(g1p[:, 4 * 256 :]),
            "w2s0": cc(w2p[:, : 4 * M]),
            "w2b1": cc(w2p[:, 4 * M : 4 * 4 * M]),
            "w2b2": cc(w2p[:, 4 * 4 * M :]),
            "w3s0": cc(w3p[:, :tmax]),
            "w3b1": cc(w3p[:, tmax : 4 * tmax]),
            "w3b2": cc(w3p[:, 4 * tmax :]),
            "xrall": xrp,
        })
    return in_maps, assign, tuple(int(v) for v in w1), tuple(int(v) for v in t), tmax


def _build_program(w1, t, tmax):
    import os as _os
    import concourse.bacc as bacc
    import concourse.mybir as mybir
    import concourse.tile as tile

    fp32 = mybir.dt.float32
    bf16 = mybir.dt.bfloat16
    RELU = mybir.ActivationFunctionType.Relu
    ABS = mybir.ActivationFunctionType.Abs
    ADD = mybir.AluOpType.add
    X = mybir.AxisListType.X

    nc = bacc.Bacc("TRN2", target_bir_lowering=False, debug=False,
                   num_devices=N_CORES)
    xT_d = nc.dram_tensor("xT", [128, B], bf16, kind="ExternalInput")
    g1s0_d = nc.dram_tensor("g1s0", [128, 256], bf16, kind="ExternalInput")
    g1b1_d = nc.dram_tensor("g1b1", [128, 3 * 256], bf16, kind="ExternalInput")
    g1b2_d = nc.dram_tensor("g1b2", [128, 4 * 256], bf16, kind="ExternalInput")
    w2s0_d = nc.dram_tensor("w2s0", [128, 4 * M], bf16, kind="ExternalInput")
    w2b1_d = nc.dram_tensor("w2b1", [128, 3 * 4 * M], bf16, kind="ExternalInput")
    w2b2_d = nc.dram_tensor("w2b2", [128, 4 * 4 * M], bf16, kind="ExternalInput")
    w3s0_d = nc.dram_tensor("w3s0", [67, tmax], bf16, kind="ExternalInput")
    w3b1_d = nc.dram_tensor("w3b1", [67, 3 * tmax], bf16, kind="ExternalInput")
    w3b2_d = nc.dram_tensor("w3b2", [67, 4 * tmax], bf16, kind="ExternalInput")
    xr_d = nc.dram_tensor("xrall", [3, JN * B], bf16, kind="ExternalInput")
    out_d = nc.dram_tensor("out", [B, JN], fp32, kind="ExternalOutput")

    with tile.TileContext(nc) as tc:
        with (
            tc.tile_pool(name="const", bufs=1) as const,
            tc.tile_pool(name="act", bufs=2) as apool,
            tc.tile_pool(name="small", bufs=1) as spool,
            tc.tile_pool(name="psa", bufs=2, space="PSUM") as pspool,
            tc.tile_pool(name="psb", bufs=2, space="PSUM") as ps3pool,
        ):
            # static SBUF tiles
            xT_t = const.tile([128, B], bf16, tag="xT")
            g1all = const.tile([128, JN * 256], bf16, tag="g1all")
            w2all = const.tile([128, JN * 4 * M], bf16, tag="w2all")
            w3all = const.tile([67, JN * tmax], bf16, tag="w3all")
            repall = const.tile([67, JN * B], bf16, tag="repall")
            c2 = spool.tile([128, 128], fp32, tag="c2")

            # ---- weight prefetch: 3 queues, contiguous sources, ordered
            # by first use. gpsimd's SWDGE is free here (descriptor
            # generation happens before any compute contends for SBUF).
            # Queue facts (measured): the sync HWDGE ring delivers only
            # ~8-10 GB/s here no matter the shape; the scalar HWDGE ring
            # does 60-85 GB/s; gpsimd SWDGE 35-130 GB/s. So: sync gets
            # only xT (first, small, needed at ~10us), scalar gets all
            # [128, .] weights, SWDGE gets the odd-shaped w3 [67, .] and
            # xr [3, .] (HWDGE serializes those onto one SDMA engine).
            nc.sync.dma_start(xT_t[:], xT_d.ap())
            nc.scalar.dma_start(g1all[:, 0:256], g1s0_d.ap())
            nc.gpsimd.dma_start(repall[64:67, :], xr_d.ap())
            nc.scalar.dma_start(w2all[:, 0 : 4 * M], w2s0_d.ap())
            nc.gpsimd.dma_start(w3all[:, 0:tmax], w3s0_d.ap())
            nc.scalar.dma_start(g1all[:, 256 : 4 * 256], g1b1_d.ap())
            nc.gpsimd.dma_start(w3all[:, tmax : 4 * tmax], w3b1_d.ap())
            nc.scalar.dma_start(w2all[:, 4 * M : 4 * 4 * M], w2b1_d.ap())
            nc.scalar.dma_start(g1all[:, 4 * 256 :], g1b2_d.ap())
            nc.gpsimd.dma_start(w3all[:, 4 * tmax :], w3b2_d.ap())
            nc.scalar.dma_start(w2all[:, 4 * 4 * M :], w2b2_d.ap())

            h1T_t = {}

            # PE warm-up: dummy matmul burst while the DMAs land. The HAM
            # clock gate opens (K=8/8, 2.4 GHz) after ~3.4us of sustained
            # PE busy; without this the whole first half runs at 1.2 GHz.
            # memset on Vector (idle), psum from the ps3 pool (first real
            # use is node 0's L3, ~2 slots in).
            warm_n = int(_os.environ.get("CTP_WARM", "8"))
            warm_s = spool.tile([64, 768], bf16, tag="warm")
            warm_ps = None
            if warm_n:
                nc.vector.memset(warm_s[:], 0)
                warm_ps = ps3pool.tile([128, 512], fp32, tag="ps3",
                                       name="warm_ps")
                for _ in range(warm_n):
                    nc.tensor.matmul(warm_ps[:], warm_s[:, 0:128],
                                     warm_s[:, 256:768], start=True, stop=True)

            def filler(k):
                # fill-phase PE keep-alive: the pipeline isn't full yet, so
                # the PE would idle waiting on the ACT chain / psum-buffer
                # recycling; any idle window re-throttles the HAM clock
                # gate to 1.2 GHz and the cold slots then take ~9us instead
                # of ~5.7 (it takes until ~t=58us to re-lock 2.4 GHz).
                # Dummy FD=512 matmuls in the FIFO bridge those known gaps.
                # FD=128: ~57ns warm / ~107ns cold each, so a misjudged
                # count displaces little real work.
                for _ in range(k):
                    if warm_ps is not None:
                        nc.tensor.matmul(warm_ps[:, 0:128], warm_s[:, 0:128],
                                         warm_s[:, 256:384], start=True,
                                         stop=True)

            def emit_l1_unit(n, pair):
                # two K=64 matmuls run concurrently in row groups (0,0) and
                # (64,0): h-chunk 2*pair in array rows 0:64, 2*pair+1 in
                # rows 64:128 (g1/xT are laid out accordingly on the host).
                g1_t = g1all[:, n * 256 : (n + 1) * 256]
                h1T = h1T_t[n]
                psA = pspool.tile([128, B], fp32, tag="ps", name=f"psA_{n}_{pair}")
                psB = pspool.tile([128, B], fp32, tag="ps", name=f"psB_{n}_{pair}")
                for bc2 in range(2):
                    nc.tensor.matmul(
                        psA[:, bc2 * 512 : (bc2 + 1) * 512],
                        g1_t[0:64, pair * 128 : (pair + 1) * 128],
                        xT_t[0:64, bc2 * 512 : (bc2 + 1) * 512],
                        start=True, stop=True, tile_position=(0, 0),
                    )
                    nc.tensor.matmul(
                        psB[:, bc2 * 512 : (bc2 + 1) * 512],
                        g1_t[64:128, pair * 128 : (pair + 1) * 128],
                        xT_t[64:128, bc2 * 512 : (bc2 + 1) * 512],
                        start=True, stop=True, tile_position=(64, 0),
                    )
                nc.scalar.activation(
                    h1T[:, (2 * pair) * B : (2 * pair + 1) * B], psA[:], RELU)
                if pair == 1 and _os.environ.get("CTP_C3S", "0") == "1":
                    # split chunk3's ACT into bc halves: L2's hc3 matmuls
                    # (the slot's chain terminal) can issue after the bc0
                    # half instead of idling ACT ~0.65us until the full
                    # [128,1024] copy completes.
                    c3 = (2 * pair + 1) * B
                    nc.scalar.activation(
                        h1T[:, c3 : c3 + 512], psB[:, 0:512], RELU)
                    nc.scalar.activation(
                        h1T[:, c3 + 512 : c3 + B], psB[:, 512:B], RELU)
                else:
                    nc.scalar.activation(
                        h1T[:, (2 * pair + 1) * B : (2 * pair + 2) * B],
                        psB[:], RELU)

            colt = _os.environ.get("CTP_COLT", "0") == "1"

            def emit_l2_mm(n, bc2, ps2, hcs):
                # default: both batch halves accumulate in ONE [64,1024]
                # tile (cols 0:512 = bc half 0 in bank k, 512:1024 = half
                # 1 in bank k+1) so r1 is a single fused ACT.
                # CTP_COLT=1: half 1 instead goes to PE column group
                # (0,64) -> psum partitions 64:128, same columns; the two
                # 4-chunk accumulations then stream CONCURRENTLY through
                # the PE (measured 2.38x for col-tiling) halving L2's PE
                # time, at the cost of two r1 ACTs (one partition-shifted).
                w2_t = w2all[:, n * 4 * M : (n + 1) * 4 * M]
                h1T = h1T_t[n]
                for hc in hcs:
                    if colt and bc2 == 1:
                        out = ps2[64:128, 0:512]
                        kw = {"tile_position": (0, 64)}
                    else:
                        out = ps2[0:64, bc2 * 512 : (bc2 + 1) * 512]
                        kw = {"tile_position": (0, 0)} if colt else {}
                    nc.tensor.matmul(
                        out,
                        w2_t[:, hc * M : (hc + 1) * M],
                        h1T[:, hc * B + bc2 * 512 : hc * B + (bc2 + 1) * 512],
                        start=(hc == 0), stop=(hc == 3), **kw,
                    )

            def emit_l2_act(n, ps2):
                if colt:
                    nc.scalar.activation(
                        repall[0:64, n * B : n * B + 512],
                        ps2[0:64, 0:512], RELU)
                    nc.scalar.activation(
                        repall[0:64, n * B + 512 : (n + 1) * B],
                        ps2[64:128, 0:512], RELU)
                else:
                    nc.scalar.activation(
                        repall[0:64, n * B : (n + 1) * B], ps2[0:64, :], RELU)

            abs_scratch = spool.tile([128, 320], bf16, tag="abs_scr")

            def emit_l3_unit(n, J, bc, on_act=False):
                w3_t = w3all[:, n * tmax : (n + 1) * tmax]
                rep = repall[:, n * B : (n + 1) * B]
                ps3 = ps3pool.tile([128, B], fp32, tag="ps3")
                stat = rep[:, bc * 128 : (bc + 1) * 128]
                nc.tensor.matmul(ps3[:, 0:512], stat, w3_t[:, 0:512],
                                 start=True, stop=True)
                nc.tensor.matmul(ps3[:, 512 : t[J]], stat, w3_t[:, 512 : t[J]],
                                 start=True, stop=True)
                col = bc * 8 + J
                if on_act:
                    # drain-step offload: ACT is idle, DVE is the drain
                    # bottleneck; Abs+accum_out computes the window sums.
                    wj = t[J] // 2
                    for s in range(2):
                        nc.scalar.activation(
                            abs_scratch[:, 0:wj],
                            ps3[:, s * wj : (s + 1) * wj], ABS,
                            accum_out=c2[:, 2 * col + s : 2 * col + s + 1])
                else:
                    nc.vector.tensor_reduce(
                        c2[:, 2 * col : 2 * col + 2],
                        ps3[:, 0 : t[J]].rearrange("p (s w) -> p s w", s=2),
                        axis=X, op=ADD, apply_absolute_value=True)

            fin_split = _os.environ.get("CTP_FIN", "1") == "1"
            gtt = _os.environ.get("CTP_GTT", "1") == "1"

            def emit_final(cols, osb_tag):
                # out[:, j] = relu(0.5*(cpos - cneg)) for osb col range
                lo, hi = cols
                c3 = c2[:, 2 * lo : 2 * hi].rearrange("p (c s) -> p c s", s=2)
                t1 = spool.tile([128, hi - lo], fp32, tag=osb_tag + "t")
                eng = nc.gpsimd if gtt else nc.vector
                eng.tensor_tensor(t1[:], c3[:, :, 0], c3[:, :, 1],
                                  op=mybir.AluOpType.subtract)
                osb = spool.tile([128, hi - lo], fp32, tag=osb_tag)
                nc.scalar.activation(osb[:], t1[:], RELU, scale=0.5)
                # SWDGE: the sync ring would take ~2us for these 16KB
                nc.gpsimd.dma_start(
                    out_d.ap().rearrange("(k p) n -> p k n", p=128)[:, lo // 8 : hi // 8, :],
                    osb[:].rearrange("p (k n) -> p k n", n=JN),
                )

            # 3-stage software pipeline: step k = L1(k) + L2(k-1) + L3(k-2).
            # Every stage consumes data finished a full step earlier, so no
            # matmul waits on a same-step ACT: the once-per-slot ~0.5us PE
            # stall of the 2-stage version re-throttled the HAM clock gate
            # (K=4/8 for 2 of every 3 windows -> PE at 1.2 GHz half the
            # time despite >93% busy).
            dof = _os.environ.get("CTP_DOF", "1") == "1"
            pipe2 = _os.environ.get("CTP_PIPE", "3") == "2"
            n_steps = JN + 1 if pipe2 else JN + 2
            for step in range(n_steps):
                if pipe2:
                    nxt = step if step < JN else None
                    cur = nxt
                    prev = step - 1 if step > 0 else None
                else:
                    nxt = step if step < JN else None
                    cur = step - 1 if 1 <= step <= JN else None
                    prev = step - 2 if step >= 2 else None
                if nxt is not None:
                    h1T_t[nxt] = apool.tile([128, 4 * B], bf16, tag="h1",
                                            name=f"h1T_{nxt}")
                drain_act = nxt is None and cur is None and dof

                def l3(k):
                    if prev is not None:
                        emit_l3_unit(prev, prev, k,
                                     on_act=drain_act and k in (3, 5, 7))
                fill_n = int(_os.environ.get("CTP_FILLS", "0"))
                if step == 0:
                    fill_n = int(_os.environ.get("CTP_FILL0", "0"))
                elif step == 1 and not pipe2:
                    fill_n = int(_os.environ.get("CTP_FILL1", "0"))
                if nxt is not None:
                    emit_l1_unit(nxt, 0)
                filler(fill_n)
                l3(0); l3(1)
                if nxt is not None:
                    emit_l1_unit(nxt, 1)
                filler(fill_n)
                l3(2); l3(3)
                if fin_split and step == n_steps - 1:
                    # drain step: node 7's bc0-3 reduces just emitted, so
                    # osb cols 0:32 are complete; ship that half early.
                    emit_final((0, 32), "osb0")
                ps2 = None
                if cur is not None:
                    if colt:
                        ps2 = pspool.tile([128, 512], fp32, tag="ps",
                                          name=f"ps2_{step}")
                        # interleave the halves' chunks so the two column
                        # groups stream through the PE concurrently
                        for hc in range(2):
                            emit_l2_mm(cur, 0, ps2, [hc])
                            emit_l2_mm(cur, 1, ps2, [hc])
                    else:
                        ps2 = pspool.tile([64, B], fp32, tag="ps",
                                          name=f"ps2_{step}")
                        emit_l2_mm(cur, 0, ps2, range(4))
                l3(4); l3(5)
                if cur is not None:
                    if colt:
                        for hc in range(2, 4):
                            emit_l2_mm(cur, 0, ps2, [hc])
                            emit_l2_mm(cur, 1, ps2, [hc])
                    else:
                        emit_l2_mm(cur, 1, ps2, range(4))
                    emit_l2_act(cur, ps2)
                filler(fill_n)
                l3(6); l3(7)
                if cur is not None and not pipe2:
                    h1T_t.pop(cur, None)
                if pipe2 and prev is not None:
                    h1T_t.pop(prev, None)

            if fin_split:
                emit_final((32, 64), "osb1")
            else:
                emit_final((0, 64), "osb")
    nc.compile()
    return nc


def _get_program(w1, t, tmax):
    import os
    key = (w1, t, tmax, os.environ.get("CTP_WARM", ""),
           os.environ.get("CTP_FIN", ""), os.environ.get("CTP_DOF", ""),
           os.environ.get("CTP_C3S", ""), os.environ.get("CTP_GTT", ""),
           os.environ.get("CTP_PIPE", ""), os.environ.get("CTP_COLT", ""),
           os.environ.get("CTP_FILL0", ""), os.environ.get("CTP_FILL1", ""), os.environ.get("CTP_FILLS", ""))
    if key not in _PROGRAM_CACHE:
        _PROGRAM_CACHE[key] = _build_program(w1, t, tmax)
    return _PROGRAM_CACHE[key]


def kernel(x, W1, W2, W3, b3, W4, b4):
    import os
    from concourse.bass_utils import run_bass_kernel_spmd

    in_maps, assign, w1, t, tmax = _prep(x, W1, W2, W3, b3, W4, b4)
    nc = _get_program(w1, t, tmax)

    trace = os.environ.get("CTP_KERNEL_TRACE", "0") == "1"
    kwargs = {}
    if trace:
        import types
        sys.path.insert(0, "/root/.axon_site")
        from trn_agent_boot.trn_boot import _ntff_profile_via_ctypes
        hook = _ntff_profile_via_ctypes("/opt/axon/libaxon_pjrt.so")
        mod = types.ModuleType("antenv.axon_hooks")
        mod.get_axon_ntff_profile_hook = lambda: hook
        mod.set_axon_ntff_profile_hook = lambda h: None
        sys.modules["antenv.axon_hooks"] = mod
        import concourse.bass_utils as bu
        bu.upload_artifacts = lambda tmpdir: f"local:{tmpdir}"
        tdir = os.environ.get("CTP_TRACE_DIR", "/tmp/ctp_trace")
        os.makedirs(tdir, exist_ok=True)
        kwargs = {"trace": True, "tmpdir": tdir}

    res = run_bass_kernel_spmd(nc, in_maps, list(range(N_CORES)), **kwargs)
    if trace:
        print(f"HW exec time: {res.exec_time_ns} ns")

    out = np.zeros((B, N_NODES), np.float32)
    for j in range(N_CORES):
        oj = np.asarray(res.results[j]["out"], np.float32)   # (B, JN)
        for J in range(JN):
            out[:, int(assign[J, j])] = oj[:, J]
    return out


# revision 22
# speedup vs baseline: 1.2065x; 1.1137x over previous
"""Trainium2 Bass kernel for nn_CausalTrajectoryPrediction.

Math (per node n of 64, batch B=1024):
    h1 = relu(x_masked @ W1[n].T)          x_masked = x with col n zeroed
    r1 = relu(h1 @ W2[n].T)
    h3 = relu([r1, x_n] @ W3[n].T + b3[n])
    out[:, n] = relu(h3 @ W4[n] + b4[n])

Restructuring (validated vs the fp32 reference on CPU):
  - The input mask folds into the weights on the host (zero the diagonal
    column of W1[n]); the "own value" path of W3 collapses to one column;
    b3 becomes a ones-row of the layer-3 stationary operand.
  - The last layer is eliminated via w*relu(z) = 0.5*(w*z + w*|z|):
        out[:, n] = 0.5 * relu(c_pos - c_neg)
    where c_pos/c_neg are DVE abs-reduces over two fixed windows of the
    layer-3 PSUM row. Columns are pre-scaled by |W4| and grouped by
    sign(W4) on the host. The linear term a = rep @ (W3ext@W4 + 2*b4*e1)
    is carried by two extra nonnegative columns (v+ in the positive
    window, v- in the negative window; rep is made elementwise >= 0 by
    splitting x_n into x+/x-), so no extra matmul or PSUM tile is needed.
  - All 8 cores share one program, but the sign-split point differs per
    node. Nodes are assigned to program slots sorted by split point; the
    few "middle" columns that are positive on some cores and negative on
    others sit inside the positive window and are duplicated at the tail
    with weight 2 (|z| - 2|z| = -|z|) on cores where they are negative.
  - Sharding: 8 nodes per core (weights are NOT replicated -> 8x less
    HBM traffic), full batch per core. Host gathers (1024, 8) per core.

v3 changes vs v2 (trace-driven, see HW model below):
  - DMA prefetch: every transfer now reads a CONTIGUOUS DRAM tensor
    (v2 sliced wide tensors; the strided reads landed all descriptors on
    ONE of the 16 SDMA engines -> 25 GB/s, weights not resident until
    t=38us, PE stalling + HAM re-throttling to 1.2 GHz mid-kernel).
    Transfers are spread over three queues (sync/scalar HWDGE + gpsimd
    SWDGE - idle before compute starts) and ordered by first use.
  - Warm-up scratch memset moved to the (otherwise idle) Vector engine
    and the warm psum to the ps3 pool, so the dummy-matmul burst starts
    right after the engine preamble (~5us) instead of ~8us, and the PE's
    HAM clock gate (default K=4/8 = 1.2 GHz; K=8/8 = 2.4 GHz after
    ~3.4us of sustained busy) is fully open when the real stream begins.
  - L2 accumulates both batch halves into ONE [64,1024] psum tile
    (different banks) and r1 is a single fused [64,1024] ACT: saves one
    352-cycle ACT instruction start per node on the critical engine.
  - Drain step: half the last node's window reduces run on ACT
    (abs+accum_out) since ACT is idle there; final combine subtract on
    gpsimd; output DMA split in halves (ship cols 0:32 early).

HW model (measured on this kernel's trace):
  ACT ACTIVATE dur ~ 304 + FD ns; DVE TENSOR_REDUCE dur ~ 307 + 1.04*FD
  (PSUM fp32 src is locked to 1 elem/cycle on both engines; GPSIMD has
  no PSUM port). Per node: ACT = 4 h1-chunk relus [128,1024] + fused r1
  = ~6.6us, DVE = 8 window-reduces [128,~550] = ~7.2us, PE (warm,
  2.4GHz) = L1 ~1.0 + L2 ~2.1 + L3 ~3.2 = ~6.3us. The slot is EW-bound
  at ~7us; everything else (DMA, PE clock, fill/drain) must stay off
  the critical path. Framework overhead is ~7.3us preamble (engine
  TENSOR_LOADs + barriers before the first DMA doorbell) + ~9.5us
  postamble (semaphore teardown) and is not controllable from here.

Set CTP_KERNEL_TRACE=1 to capture a neuron-profile trace and print
"HW exec time: <ns> ns".
"""
import sys

sys.path.insert(0, "/opt/trn_rl_repo")

import numpy as np
import ml_dtypes

N_NODES = 64
H = 512
B = 1024
M = 64
N_CORES = 8
JN = 8           # nodes (slots) per core
BC = 8           # batch chunks of 128
BF16 = ml_dtypes.bfloat16

_PROGRAM_CACHE = {}


def _prep(x, W1, W2, W3, b3, W4, b4):
    """Build per-core input maps + program-shape metadata."""
    x = np.asarray(x, np.float32)
    W1 = np.asarray(W1, np.float32)
    W2 = np.asarray(W2, np.float32)
    W3 = np.asarray(W3, np.float32)
    b3 = np.asarray(b3, np.float32)
    W4 = np.asarray(W4, np.float32)
    b4 = np.asarray(b4, np.float32)

    ppos = (W4 >= 0).sum(axis=1)            # sign-split point per node
    order = np.argsort(ppos, kind="stable")
    assign = order.reshape(JN, N_CORES)     # assign[J, core] -> node id
    pmax = np.array([int(ppos[assign[J]].max()) for J in range(JN)])
    pmin = np.array([int(ppos[assign[J]].min()) for J in range(JN)])
    assert pmax.max() < 512, "degenerate all-positive W4 row not supported"
    # symmetric reduce windows: [0:w1) abs+, [w1:2*w1) abs- (zero padded).
    w1 = np.maximum(1 + pmax, 513 - pmin)
    t = 2 * w1                              # per-slot moving width
    # Slot order is arbitrary: run the widest-t group first (its extra
    # reduce width overlaps pipeline fill) and the narrowest last (the
    # drain step's 8 serial reduces scale with t of the final slot).
    perm = np.argsort(-t, kind="stable")
    assign = assign[perm]
    w1 = w1[perm]
    t = t[perm]
    pmax = pmax[perm]
    pmin = pmin[perm]
    tmax = int(t.max())

    xT1 = np.ascontiguousarray(x.T).astype(BF16)         # (64, 1024)
    xT = np.concatenate([xT1, xT1], axis=0)              # (128, 1024) doubled
    in_maps = []
    for j in range(N_CORES):
        g1 = np.zeros((JN, 128, 256), BF16)
        w2 = np.zeros((JN, 128, 4 * M), BF16)
        w3 = np.zeros((JN, 128, tmax), BF16)
        xr = np.zeros((JN, 3, B), BF16)
        for J in range(JN):
            n = int(assign[J, j])
            P = int(ppos[n])
            g1t = W1[n].T.copy()                          # (64 i, 512 h)
            g1t[n, :] = 0.0
            for pair in range(2):
                g1[J][0:64, pair * 128 : (pair + 1) * 128] = \
                    g1t[:, (2 * pair) * 128 : (2 * pair + 1) * 128].astype(BF16)
                g1[J][64:128, pair * 128 : (pair + 1) * 128] = \
                    g1t[:, (2 * pair + 1) * 128 : (2 * pair + 2) * 128].astype(BF16)
            w2t = W2[n].T                                 # (512 h, 64 m)
            w2[J] = np.ascontiguousarray(
                w2t.reshape(4, 128, M).transpose(1, 0, 2).reshape(128, 4 * M)
            ).astype(BF16)

            w4 = W4[n]
            w3ext = np.zeros((66, H), np.float32)
            w3ext[:64] = W3[n, :, :64].T
            w3ext[64] = W3[n, :, 64 + n]
            w3ext[65] = b3[n]
            scaled = w3ext * np.abs(w4)[None, :]
            pos = np.where(w4 >= 0)[0]
            neg = np.where(w4 < 0)[0]
            nmid = pmax[J] - P
            midc, certain = neg[:nmid], neg[nmid:]

            def lift(c):                                  # (66,k) -> (67,k)
                o = np.zeros((67, c.shape[1]), np.float32)
                o[:64] = c[:64]
                o[64] = c[64]
                o[65] = -c[64]
                o[66] = c[65]
                return o

            v = (w3ext @ w4).astype(np.float32)
            v[65] += 2.0 * b4[n]
            v67 = np.zeros(67, np.float32)
            v67[:64] = v[:64]
            v67[64] = v[64]
            v67[65] = -v[64]
            v67[66] = v[65]

            pad = np.zeros((67, tmax), np.float32)
            pad[:, 0] = np.maximum(v67, 0)                       # colA
            pad[:, 1 : 1 + P] = lift(scaled[:, pos])
            pad[:, 1 + P : 1 + pmax[J]] = lift(scaled[:, midc])
            nc_ = len(certain)
            pad[:, w1[J] : w1[J] + nc_] = lift(scaled[:, certain])
            pad[:, w1[J] + nc_ : w1[J] + nc_ + nmid] = 2.0 * lift(scaled[:, midc])
            pad[:, w1[J] + nc_ + nmid] = np.maximum(-v67, 0)     # colB
            w3[J][:67] = pad.astype(BF16)

            xr[J, 0] = np.maximum(x[:, n], 0).astype(BF16)
            xr[J, 1] = np.maximum(-x[:, n], 0).astype(BF16)
            xr[J, 2] = 1.0
        # partition-major packing: node n at free-axis cols [n*w : (n+1)*w]
        g1p = np.ascontiguousarray(g1.transpose(1, 0, 2).reshape(128, JN * 256))
        w2p = np.ascontiguousarray(w2.transpose(1, 0, 2).reshape(128, JN * 4 * M))
        w3p = np.ascontiguousarray(w3.transpose(1, 0, 2).reshape(128, JN * tmax))
        xrp = np.ascontiguousarray(xr.transpose(1, 0, 2).reshape(3, JN * B))
        # every DMA below reads one of these CONTIGUOUS arrays end-to-end
        # (a strided DRAM read serializes onto one SDMA engine: ~25 GB/s)
        cc = lambda a: np.ascontiguousarray(a)
        in_maps.append({
            "xT": xT,
            "g1s0": cc(g1p[:, :256]),
            "g1b1": cc(g1p[:, 256 : 4 * 256]),
            "g1b2": cc(g1p[:, 4 * 256 :]),
            "w2s0": cc(w2p[:, : 4 * M]),
            "w2b1": cc(w2p[:, 4 * M : 4 * 4 * M]),
            "w2b2": cc(w2p[:, 4 * 4 * M :]),
            "w3s0": cc(w3p[:, :tmax]),
            "w3b1": cc(w3p[:, tmax : 4 * tmax]),
            "w3b2": cc(w3p[:, 4 * tmax :]),
            "xrall": xrp,
        })
    return in_maps, assign, tuple(int(v) for v in w1), tuple(int(v) for v in t), tmax


def _build_program(w1, t, tmax):
    import os as _os
    import concourse.bacc as bacc
    import concourse.mybir as mybir
    import concourse.tile as tile

    fp32 = mybir.dt.float32
    bf16 = mybir.dt.bfloat16
    RELU = mybir.ActivationFunctionType.Relu
    ABS = mybir.ActivationFunctionType.Abs
    ADD = mybir.AluOpType.add
    X = mybir.AxisListType.X

    nc = bacc.Bacc("TRN2", target_bir_lowering=False, debug=False,
                   num_devices=N_CORES)
    xT_d = nc.dram_tensor("xT", [128, B], bf16, kind="ExternalInput")
    g1s0_d = nc.dram_tensor("g1s0", [128, 256], bf16, kind="ExternalInput")
    g1b1_d = nc.dram_tensor("g1b1", [128, 3 * 256], bf16, kind="ExternalInput")
    g1b2_d = nc.dram_tensor("g1b2", [128, 4 * 256], bf16, kind="ExternalInput")
    w2s0_d = nc.dram_tensor("w2s0", [128, 4 * M], bf16, kind="ExternalInput")
    w2b1_d = nc.dram_tensor("w2b1", [128, 3 * 4 * M], bf16, kind="ExternalInput")
    w2b2_d = nc.dram_tensor("w2b2", [128, 4 * 4 * M], bf16, kind="ExternalInput")
    w3s0_d = nc.dram_tensor("w3s0", [128, tmax], bf16, kind="ExternalInput")
    w3b1_d = nc.dram_tensor("w3b1", [128, 3 * tmax], bf16, kind="ExternalInput")
    w3b2_d = nc.dram_tensor("w3b2", [128, 4 * tmax], bf16, kind="ExternalInput")
    xr_d = nc.dram_tensor("xrall", [3, JN * B], bf16, kind="ExternalInput")
    out_d = nc.dram_tensor("out", [B, JN], fp32, kind="ExternalOutput")

    with tile.TileContext(nc) as tc:
        with (
            tc.tile_pool(name="const", bufs=1) as const,
            tc.tile_pool(name="act", bufs=2) as apool,
            tc.tile_pool(name="small", bufs=1) as spool,
            tc.tile_pool(name="psa", bufs=2, space="PSUM") as pspool,
            tc.tile_pool(name="psb", bufs=2, space="PSUM") as ps3pool,
        ):
            # static SBUF tiles
            xT_t = const.tile([128, B], bf16, tag="xT")
            g1all = const.tile([128, JN * 256], bf16, tag="g1all")
            w2all = const.tile([128, JN * 4 * M], bf16, tag="w2all")
            w3all = const.tile([128, JN * tmax], bf16, tag="w3all")
            repall = const.tile([128, JN * B], bf16, tag="repall")
            c2 = spool.tile([128, 128], fp32, tag="c2")

            # ---- weight prefetch: 3 queues, contiguous sources, ordered
            # by first use. gpsimd's SWDGE is free here (descriptor
            # generation happens before any compute contends for SBUF).
            # Queue facts (measured): the sync HWDGE ring delivers only
            # ~8-10 GB/s here no matter the shape; the scalar HWDGE ring
            # does 60-85 GB/s; gpsimd SWDGE 35-130 GB/s. So: sync gets
            # only xT (first, small, needed at ~10us), scalar gets all
            # [128, .] weights, SWDGE gets the odd-shaped w3 [67, .] and
            # xr [3, .] (HWDGE serializes those onto one SDMA engine).
            nc.sync.dma_start(xT_t[:], xT_d.ap())
            nc.scalar.dma_start(g1all[:, 0:256], g1s0_d.ap())
            nc.vector.memset(repall[64:128, :], 0)
            nc.gpsimd.dma_start(repall[64:67, :], xr_d.ap())
            nc.scalar.dma_start(w2all[:, 0 : 4 * M], w2s0_d.ap())
            nc.gpsimd.dma_start(w3all[:, 0:tmax], w3s0_d.ap())
            nc.scalar.dma_start(g1all[:, 256 : 4 * 256], g1b1_d.ap())
            nc.gpsimd.dma_start(w3all[:, tmax : 4 * tmax], w3b1_d.ap())
            nc.scalar.dma_start(w2all[:, 4 * M : 4 * 4 * M], w2b1_d.ap())
            nc.scalar.dma_start(g1all[:, 4 * 256 :], g1b2_d.ap())
            nc.gpsimd.dma_start(w3all[:, 4 * tmax :], w3b2_d.ap())
            nc.scalar.dma_start(w2all[:, 4 * 4 * M :], w2b2_d.ap())

            h1T_t = {}

            # PE warm-up: dummy matmul burst while the DMAs land. The HAM
            # clock gate opens (K=8/8, 2.4 GHz) after ~3.4us of sustained
            # PE busy; without this the whole first half runs at 1.2 GHz.
            # memset on Vector (idle), psum from the ps3 pool (first real
            # use is node 0's L3, ~2 slots in).
            warm_n = int(_os.environ.get("CTP_WARM", "8"))
            warm_s = spool.tile([64, 768], bf16, tag="warm")
            warm_ps = None
            if warm_n:
                nc.vector.memset(warm_s[:], 0)
                warm_ps = ps3pool.tile([128, 512], fp32, tag="ps3",
                                       name="warm_ps")
                for _ in range(warm_n):
                    nc.tensor.matmul(warm_ps[:], warm_s[:, 0:128],
                                     warm_s[:, 256:768], start=True, stop=True)

            def filler(k):
                # fill-phase PE keep-alive: the pipeline isn't full yet, so
                # the PE would idle waiting on the ACT chain / psum-buffer
                # recycling; any idle window re-throttles the HAM clock
                # gate to 1.2 GHz and the cold slots then take ~9us instead
                # of ~5.7 (it takes until ~t=58us to re-lock 2.4 GHz).
                # Dummy FD=512 matmuls in the FIFO bridge those known gaps.
                # FD=128: ~57ns warm / ~107ns cold each, so a misjudged
                # count displaces little real work.
                for _ in range(k):
                    if warm_ps is not None:
                        nc.tensor.matmul(warm_ps[:, 0:128], warm_s[:, 0:128],
                                         warm_s[:, 256:384], start=True,
                                         stop=True)

            def emit_l1_unit(n, pair):
                # two K=64 matmuls run concurrently in row groups (0,0) and
                # (64,0): h-chunk 2*pair in array rows 0:64, 2*pair+1 in
                # rows 64:128 (g1/xT are laid out accordingly on the host).
                g1_t = g1all[:, n * 256 : (n + 1) * 256]
                h1T = h1T_t[n]
                psA = pspool.tile([128, B], fp32, tag="ps", name=f"psA_{n}_{pair}")
                psB = pspool.tile([128, B], fp32, tag="ps", name=f"psB_{n}_{pair}")

                def mmA(bc2):
                    nc.tensor.matmul(
                        psA[:, bc2 * 512 : (bc2 + 1) * 512],
                        g1_t[0:64, pair * 128 : (pair + 1) * 128],
                        xT_t[0:64, bc2 * 512 : (bc2 + 1) * 512],
                        start=True, stop=True, tile_position=(0, 0),
                    )

                def mmB(bc2):
                    nc.tensor.matmul(
                        psB[:, bc2 * 512 : (bc2 + 1) * 512],
                        g1_t[64:128, pair * 128 : (pair + 1) * 128],
                        xT_t[64:128, bc2 * 512 : (bc2 + 1) * 512],
                        start=True, stop=True, tile_position=(64, 0),
                    )
                # A0,B0,B1,A1: the redundant 2nd LDWEIGHTS of each row
                # group waits for that group's 1st matmul to finish; this
                # order lets it load while the OTHER group streams, so the
                # two halves of each pair overlap fully.
                mmA(0); mmB(0); mmB(1); mmA(1)
                nc.scalar.activation(
                    h1T[:, (2 * pair) * B : (2 * pair + 1) * B], psA[:], RELU)
                if pair == 1 and _os.environ.get("CTP_C3S", "0") == "1":
                    # split chunk3's ACT into bc halves: L2's hc3 matmuls
                    # (the slot's chain terminal) can issue after the bc0
                    # half instead of idling ACT ~0.65us until the full
                    # [128,1024] copy completes.
                    c3 = (2 * pair + 1) * B
                    nc.scalar.activation(
                        h1T[:, c3 : c3 + 512], psB[:, 0:512], RELU)
                    nc.scalar.activation(
                        h1T[:, c3 + 512 : c3 + B], psB[:, 512:B], RELU)
                else:
                    nc.scalar.activation(
                        h1T[:, (2 * pair + 1) * B : (2 * pair + 2) * B],
                        psB[:], RELU)

            colt = _os.environ.get("CTP_COLT", "0") == "1"

            def emit_l2_mm(n, bc2, ps2, hcs):
                # default: both batch halves accumulate in ONE [64,1024]
                # tile (cols 0:512 = bc half 0 in bank k, 512:1024 = half
                # 1 in bank k+1) so r1 is a single fused ACT.
                # CTP_COLT=1: half 1 instead goes to PE column group
                # (0,64) -> psum partitions 64:128, same columns; the two
                # 4-chunk accumulations then stream CONCURRENTLY through
                # the PE (measured 2.38x for col-tiling) halving L2's PE
                # time, at the cost of two r1 ACTs (one partition-shifted).
                w2_t = w2all[:, n * 4 * M : (n + 1) * 4 * M]
                h1T = h1T_t[n]
                for hc in hcs:
                    if colt and bc2 == 1:
                        out = ps2[64:128, 0:512]
                        kw = {"tile_position": (0, 64)}
                    else:
                        out = ps2[0:64, bc2 * 512 : (bc2 + 1) * 512]
                        kw = {"tile_position": (0, 0)} if colt else {}
                    nc.tensor.matmul(
                        out,
                        w2_t[:, hc * M : (hc + 1) * M],
                        h1T[:, hc * B + bc2 * 512 : hc * B + (bc2 + 1) * 512],
                        start=(hc == 0), stop=(hc == 3), **kw,
                    )

            def emit_l2_act(n, ps2):
                if colt:
                    nc.scalar.activation(
                        repall[0:64, n * B : n * B + 512],
                        ps2[0:64, 0:512], RELU)
                    nc.scalar.activation(
                        repall[0:64, n * B + 512 : (n + 1) * B],
                        ps2[64:128, 0:512], RELU)
                else:
                    nc.scalar.activation(
                        repall[0:64, n * B : (n + 1) * B], ps2[0:64, :], RELU)

            abs_scratch = spool.tile([128, 320], bf16, tag="abs_scr")

            def emit_l3_unit(n, J, bc, on_act=False):
                w3_t = w3all[:, n * tmax : (n + 1) * tmax]
                rep = repall[:, n * B : (n + 1) * B]
                ps3 = ps3pool.tile([128, B], fp32, tag="ps3")
                stat = rep[:, bc * 128 : (bc + 1) * 128]
                nc.tensor.matmul(ps3[:, 0:512], stat, w3_t[:, 0:512],
                                 start=True, stop=True)
                nc.tensor.matmul(ps3[:, 512 : t[J]], stat, w3_t[:, 512 : t[J]],
                                 start=True, stop=True)
                col = bc * 8 + J
                if on_act:
                    # drain-step offload: ACT is idle, DVE is the drain
                    # bottleneck; Abs+accum_out computes the window sums.
                    wj = t[J] // 2
                    for s in range(2):
                        nc.scalar.activation(
                            abs_scratch[:, 0:wj],
                            ps3[:, s * wj : (s + 1) * wj], ABS,
                            accum_out=c2[:, 2 * col + s : 2 * col + s + 1])
                else:
                    nc.vector.tensor_reduce(
                        c2[:, 2 * col : 2 * col + 2],
                        ps3[:, 0 : t[J]].rearrange("p (s w) -> p s w", s=2),
                        axis=X, op=ADD, apply_absolute_value=True)

            fin_split = _os.environ.get("CTP_FIN", "1") == "1"
            gtt = _os.environ.get("CTP_GTT", "1") == "1"

            def emit_final(cols, osb_tag):
                # out[:, j] = relu(0.5*(cpos - cneg)) for osb col range
                lo, hi = cols
                c3 = c2[:, 2 * lo : 2 * hi].rearrange("p (c s) -> p c s", s=2)
                t1 = spool.tile([128, hi - lo], fp32, tag=osb_tag + "t")
                eng = nc.gpsimd if gtt else nc.vector
                eng.tensor_tensor(t1[:], c3[:, :, 0], c3[:, :, 1],
                                  op=mybir.AluOpType.subtract)
                osb = spool.tile([128, hi - lo], fp32, tag=osb_tag)
                nc.scalar.activation(osb[:], t1[:], RELU, scale=0.5)
                # SWDGE: the sync ring would take ~2us for these 16KB
                nc.gpsimd.dma_start(
                    out_d.ap().rearrange("(k p) n -> p k n", p=128)[:, lo // 8 : hi // 8, :],
                    osb[:].rearrange("p (k n) -> p k n", n=JN),
                )

            # 3-stage software pipeline: step k = L1(k) + L2(k-1) + L3(k-2).
            # Every stage consumes data finished a full step earlier, so no
            # matmul waits on a same-step ACT: the once-per-slot ~0.5us PE
            # stall of the 2-stage version re-throttled the HAM clock gate
            # (K=4/8 for 2 of every 3 windows -> PE at 1.2 GHz half the
            # time despite >93% busy).
            dof = _os.environ.get("CTP_DOF", "1") == "1"
            pipe2 = _os.environ.get("CTP_PIPE", "3") == "2"
            n_steps = JN + 1 if pipe2 else JN + 2
            for step in range(n_steps):
                if pipe2:
                    nxt = step if step < JN else None
                    cur = nxt
                    prev = step - 1 if step > 0 else None
                else:
                    nxt = step if step < JN else None
                    cur = step - 1 if 1 <= step <= JN else None
                    prev = step - 2 if step >= 2 else None
                if nxt is not None:
                    h1T_t[nxt] = apool.tile([128, 4 * B], bf16, tag="h1",
                                            name=f"h1T_{nxt}")
                drain_act = nxt is None and cur is None and dof

                def l3(k):
                    if prev is not None:
                        emit_l3_unit(prev, prev, k,
                                     on_act=drain_act and k in (3, 5, 7))
                fill_n = int(_os.environ.get("CTP_FILLS", "0"))
                if step == 0:
                    fill_n = int(_os.environ.get("CTP_FILL0", "0"))
                elif step == 1 and not pipe2:
                    fill_n = int(_os.environ.get("CTP_FILL1", "0"))
                if nxt is not None:
                    emit_l1_unit(nxt, 0)
                filler(fill_n)
                l3(0); l3(1)
                if nxt is not None:
                    emit_l1_unit(nxt, 1)
                filler(fill_n)
                l3(2); l3(3)
                if fin_split and step == n_steps - 1:
                    # drain step: node 7's bc0-3 reduces just emitted, so
                    # osb cols 0:32 are complete; ship that half early.
                    emit_final((0, 32), "osb0")
                ps2 = None
                if cur is not None:
                    if colt:
                        ps2 = pspool.tile([128, 512], fp32, tag="ps",
                                          name=f"ps2_{step}")
                        # interleave the halves' chunks so the two column
                        # groups stream through the PE concurrently
                        for hc in range(2):
                            emit_l2_mm(cur, 0, ps2, [hc])
                            emit_l2_mm(cur, 1, ps2, [hc])
                    else:
                        ps2 = pspool.tile([64, B], fp32, tag="ps",
                                          name=f"ps2_{step}")
                        emit_l2_mm(cur, 0, ps2, range(4))
                l3(4); l3(5)
                if cur is not None:
                    if colt:
                        for hc in range(2, 4):
                            emit_l2_mm(cur, 0, ps2, [hc])
                            emit_l2_mm(cur, 1, ps2, [hc])
                    else:
                        emit_l2_mm(cur, 1, ps2, range(4))
                    emit_l2_act(cur, ps2)
                filler(fill_n)
                l3(6); l3(7)
                if cur is not None and not pipe2:
                    h1T_t.pop(cur, None)
                if pipe2 and prev is not None:
                    h1T_t.pop(prev, None)

            if fin_split:
                emit_final((32, 64), "osb1")
            else:
                emit_final((0, 64), "osb")
    nc.compile()
    return nc


def _get_program(w1, t, tmax):
    import os
    key = (w1, t, tmax, os.environ.get("CTP_WARM", ""),
           os.environ.get("CTP_FIN", ""), os.environ.get("CTP_DOF", ""),
           os.environ.get("CTP_C3S", ""), os.environ.get("CTP_GTT", ""),
           os.environ.get("CTP_PIPE", ""), os.environ.get("CTP_COLT", ""),
           os.environ.get("CTP_FILL0", ""), os.environ.get("CTP_FILL1", ""), os.environ.get("CTP_FILLS", ""))
    if key not in _PROGRAM_CACHE:
        _PROGRAM_CACHE[key] = _build_program(w1, t, tmax)
    return _PROGRAM_CACHE[key]


def kernel(x, W1, W2, W3, b3, W4, b4):
    import os
    from concourse.bass_utils import run_bass_kernel_spmd

    in_maps, assign, w1, t, tmax = _prep(x, W1, W2, W3, b3, W4, b4)
    nc = _get_program(w1, t, tmax)

    trace = os.environ.get("CTP_KERNEL_TRACE", "0") == "1"
    kwargs = {}
    if trace:
        import types
        sys.path.insert(0, "/root/.axon_site")
        from trn_agent_boot.trn_boot import _ntff_profile_via_ctypes
        hook = _ntff_profile_via_ctypes("/opt/axon/libaxon_pjrt.so")
        mod = types.ModuleType("antenv.axon_hooks")
        mod.get_axon_ntff_profile_hook = lambda: hook
        mod.set_axon_ntff_profile_hook = lambda h: None
        sys.modules["antenv.axon_hooks"] = mod
        import concourse.bass_utils as bu
        bu.upload_artifacts = lambda tmpdir: f"local:{tmpdir}"
        tdir = os.environ.get("CTP_TRACE_DIR", "/tmp/ctp_trace")
        os.makedirs(tdir, exist_ok=True)
        kwargs = {"trace": True, "tmpdir": tdir}

    res = run_bass_kernel_spmd(nc, in_maps, list(range(N_CORES)), **kwargs)
    if trace:
        print(f"HW exec time: {res.exec_time_ns} ns")

    out = np.zeros((B, N_NODES), np.float32)
    for j in range(N_CORES):
        oj = np.asarray(res.results[j]["out"], np.float32)   # (B, JN)
        for J in range(JN):
            out[:, int(assign[J, j])] = oj[:, J]
    return out


# revision 25
# speedup vs baseline: 1.2590x; 1.0435x over previous
"""Trainium2 Bass kernel for nn_CausalTrajectoryPrediction.

Math (per node n of 64, batch B=1024):
    h1 = relu(x_masked @ W1[n].T)          x_masked = x with col n zeroed
    r1 = relu(h1 @ W2[n].T)
    h3 = relu([r1, x_n] @ W3[n].T + b3[n])
    out[:, n] = relu(h3 @ W4[n] + b4[n])

Restructuring (validated vs the fp32 reference on CPU):
  - The input mask folds into the weights on the host (zero the diagonal
    column of W1[n]); the "own value" path of W3 collapses to one column;
    b3 becomes a ones-row of the layer-3 stationary operand.
  - The last layer is eliminated via w*relu(z) = 0.5*(w*z + w*|z|):
        out[:, n] = 0.5 * relu(c_pos - c_neg)
    where c_pos/c_neg are DVE abs-reduces over two fixed windows of the
    layer-3 PSUM row. Columns are pre-scaled by |W4| and grouped by
    sign(W4) on the host. The linear term a = rep @ (W3ext@W4 + 2*b4*e1)
    is carried by two extra nonnegative columns (v+ in the positive
    window, v- in the negative window; rep is made elementwise >= 0 by
    splitting x_n into x+/x-), so no extra matmul or PSUM tile is needed.
  - All 8 cores share one program, but the sign-split point differs per
    node. Nodes are assigned to program slots sorted by split point; the
    few "middle" columns that are positive on some cores and negative on
    others sit inside the positive window and are duplicated at the tail
    with weight 2 (|z| - 2|z| = -|z|) on cores where they are negative.
  - Sharding: 8 nodes per core (weights are NOT replicated -> 8x less
    HBM traffic), full batch per core. Host gathers (1024, 8) per core.

v3 changes vs v2 (trace-driven, see HW model below):
  - DMA prefetch: every transfer now reads a CONTIGUOUS DRAM tensor
    (v2 sliced wide tensors; the strided reads landed all descriptors on
    ONE of the 16 SDMA engines -> 25 GB/s, weights not resident until
    t=38us, PE stalling + HAM re-throttling to 1.2 GHz mid-kernel).
    Transfers are spread over three queues (sync/scalar HWDGE + gpsimd
    SWDGE - idle before compute starts) and ordered by first use.
  - Warm-up scratch memset moved to the (otherwise idle) Vector engine
    and the warm psum to the ps3 pool, so the dummy-matmul burst starts
    right after the engine preamble (~5us) instead of ~8us, and the PE's
    HAM clock gate (default K=4/8 = 1.2 GHz; K=8/8 = 2.4 GHz after
    ~3.4us of sustained busy) is fully open when the real stream begins.
  - L2 accumulates both batch halves into ONE [64,1024] psum tile
    (different banks) and r1 is a single fused [64,1024] ACT: saves one
    352-cycle ACT instruction start per node on the critical engine.
  - Drain step: half the last node's window reduces run on ACT
    (abs+accum_out) since ACT is idle there; final combine subtract on
    gpsimd; output DMA split in halves (ship cols 0:32 early).

HW model (measured on this kernel's trace):
  ACT ACTIVATE dur ~ 304 + FD ns; DVE TENSOR_REDUCE dur ~ 307 + 1.04*FD
  (PSUM fp32 src is locked to 1 elem/cycle on both engines; GPSIMD has
  no PSUM port). Per node: ACT = 4 h1-chunk relus [128,1024] + fused r1
  = ~6.6us, DVE = 8 window-reduces [128,~550] = ~7.2us, PE (warm,
  2.4GHz) = L1 ~1.0 + L2 ~2.1 + L3 ~3.2 = ~6.3us. The slot is EW-bound
  at ~7us; everything else (DMA, PE clock, fill/drain) must stay off
  the critical path. Framework overhead is ~7.3us preamble (engine
  TENSOR_LOADs + barriers before the first DMA doorbell) + ~9.5us
  postamble (semaphore teardown) and is not controllable from here.

Set CTP_KERNEL_TRACE=1 to capture a neuron-profile trace and print
"HW exec time: <ns> ns".
"""
import sys

sys.path.insert(0, "/opt/trn_rl_repo")

import numpy as np
import ml_dtypes

N_NODES = 64
H = 512
B = 1024
M = 64
N_CORES = 8
JN = 8           # nodes (slots) per core
BC = 8           # batch chunks of 128
BF16 = ml_dtypes.bfloat16

_PROGRAM_CACHE = {}


def _prep(x, W1, W2, W3, b3, W4, b4):
    """Build per-core input maps + program-shape metadata."""
    x = np.asarray(x, np.float32)
    W1 = np.asarray(W1, np.float32)
    W2 = np.asarray(W2, np.float32)
    W3 = np.asarray(W3, np.float32)
    b3 = np.asarray(b3, np.float32)
    W4 = np.asarray(W4, np.float32)
    b4 = np.asarray(b4, np.float32)

    ppos = (W4 >= 0).sum(axis=1)            # sign-split point per node
    order = np.argsort(ppos, kind="stable")
    assign = order.reshape(JN, N_CORES)     # assign[J, core] -> node id
    pmax = np.array([int(ppos[assign[J]].max()) for J in range(JN)])
    pmin = np.array([int(ppos[assign[J]].min()) for J in range(JN)])
    assert pmax.max() < 512, "degenerate all-positive W4 row not supported"
    # symmetric reduce windows: [0:w1) abs+, [w1:2*w1) abs- (zero padded).
    w1 = np.maximum(1 + pmax, 513 - pmin)
    t = 2 * w1                              # per-slot moving width
    # Slot order is arbitrary: run the widest-t group first (its extra
    # reduce width overlaps pipeline fill) and the narrowest last (the
    # drain step's 8 serial reduces scale with t of the final slot).
    perm = np.argsort(-t, kind="stable")
    assign = assign[perm]
    w1 = w1[perm]
    t = t[perm]
    pmax = pmax[perm]
    pmin = pmin[perm]
    tmax = int(t.max())

    xT1 = np.ascontiguousarray(x.T).astype(BF16)         # (64, 1024)
    xT = np.concatenate([xT1, xT1], axis=0)              # (128, 1024) doubled
    in_maps = []
    for j in range(N_CORES):
        g1 = np.zeros((JN, 128, 256), BF16)
        w2 = np.zeros((JN, 128, 4 * M), BF16)
        w3 = np.zeros((JN, 128, tmax), BF16)
        xr = np.zeros((JN, 3, B), BF16)
        for J in range(JN):
            n = int(assign[J, j])
            P = int(ppos[n])
            g1t = W1[n].T.copy()                          # (64 i, 512 h)
            g1t[n, :] = 0.0
            for pair in range(2):
                g1[J][0:64, pair * 128 : (pair + 1) * 128] = \
                    g1t[:, (2 * pair) * 128 : (2 * pair + 1) * 128].astype(BF16)
                g1[J][64:128, pair * 128 : (pair + 1) * 128] = \
                    g1t[:, (2 * pair + 1) * 128 : (2 * pair + 2) * 128].astype(BF16)
            w2t = W2[n].T                                 # (512 h, 64 m)
            w2[J] = np.ascontiguousarray(
                w2t.reshape(4, 128, M).transpose(1, 0, 2).reshape(128, 4 * M)
            ).astype(BF16)

            w4 = W4[n]
            w3ext = np.zeros((66, H), np.float32)
            w3ext[:64] = W3[n, :, :64].T
            w3ext[64] = W3[n, :, 64 + n]
            w3ext[65] = b3[n]
            scaled = w3ext * np.abs(w4)[None, :]
            pos = np.where(w4 >= 0)[0]
            neg = np.where(w4 < 0)[0]
            nmid = pmax[J] - P
            midc, certain = neg[:nmid], neg[nmid:]

            def lift(c):                                  # (66,k) -> (67,k)
                o = np.zeros((67, c.shape[1]), np.float32)
                o[:64] = c[:64]
                o[64] = c[64]
                o[65] = -c[64]
                o[66] = c[65]
                return o

            v = (w3ext @ w4).astype(np.float32)
            v[65] += 2.0 * b4[n]
            v67 = np.zeros(67, np.float32)
            v67[:64] = v[:64]
            v67[64] = v[64]
            v67[65] = -v[64]
            v67[66] = v[65]

            pad = np.zeros((67, tmax), np.float32)
            pad[:, 0] = np.maximum(v67, 0)                       # colA
            pad[:, 1 : 1 + P] = lift(scaled[:, pos])
            pad[:, 1 + P : 1 + pmax[J]] = lift(scaled[:, midc])
            nc_ = len(certain)
            pad[:, w1[J] : w1[J] + nc_] = lift(scaled[:, certain])
            pad[:, w1[J] + nc_ : w1[J] + nc_ + nmid] = 2.0 * lift(scaled[:, midc])
            pad[:, w1[J] + nc_ + nmid] = np.maximum(-v67, 0)     # colB
            w3[J][:67] = pad.astype(BF16)

            xr[J, 0] = np.maximum(x[:, n], 0).astype(BF16)
            xr[J, 1] = np.maximum(-x[:, n], 0).astype(BF16)
            xr[J, 2] = 1.0
        # partition-major packing: node n at free-axis cols [n*w : (n+1)*w]
        g1p = np.ascontiguousarray(g1.transpose(1, 0, 2).reshape(128, JN * 256))
        w2p = np.ascontiguousarray(w2.transpose(1, 0, 2).reshape(128, JN * 4 * M))
        w3p = np.ascontiguousarray(w3.transpose(1, 0, 2).reshape(128, JN * tmax))
        xrp = np.ascontiguousarray(xr.transpose(1, 0, 2).reshape(3, JN * B))
        # every DMA below reads one of these CONTIGUOUS arrays end-to-end
        # (a strided DRAM read serializes onto one SDMA engine: ~25 GB/s)
        cc = lambda a: np.ascontiguousarray(a)
        in_maps.append({
            "xT": xT,
            "g1s0": cc(g1p[:, :256]),
            "g1b1": cc(g1p[:, 256 : 4 * 256]),
            "g1b2": cc(g1p[:, 4 * 256 :]),
            "w2s0": cc(w2p[:, : 4 * M]),
            "w2b1": cc(w2p[:, 4 * M : 4 * 4 * M]),
            "w2b2": cc(w2p[:, 4 * 4 * M :]),
            "w3s0": cc(w3p[:, :tmax]),
            "w3b1": cc(w3p[:, tmax : 4 * tmax]),
            "w3b2": cc(w3p[:, 4 * tmax :]),
            "xrall": xrp,
        })
    return in_maps, assign, tuple(int(v) for v in w1), tuple(int(v) for v in t), tmax


def _patch_ldw_opt():
    # walrus's LDWEIGHTS-dedup pass is off by default in bass_utils; our
    # L3 issues two matmuls per bc chunk with the SAME stationary, so the
    # redundant second load (~107ns x 64/core) is pure PE time. Enable the
    # pass (numerics are verified by the caller's rel-err check).
    import os as _os
    if _os.environ.get("CTP_LDWOPT", "0") != "1":
        return
    import concourse.bass_utils as _bu
    import inspect as _inspect
    srcf = _inspect.getsourcefile(_bu)
    # patch the module constant in memory only: run_command sees the list
    # built inside compile_bir_to_neff; easiest robust hook is wrapping
    # run_command to rewrite the flag.
    if getattr(_bu, "_ctp_ldw_patched", False):
        return
    _orig = _bu.run_command

    def _patched(cmd, *a, **kw):
        cmd = [c.replace("--enable-ldw-opt=false", "--enable-ldw-opt=true")
               if isinstance(c, str) else c for c in cmd]
        return _orig(cmd, *a, **kw)

    _bu.run_command = _patched
    _bu._ctp_ldw_patched = True


def _build_program(w1, t, tmax):
    import os as _os
    _patch_ldw_opt()
    import concourse.bacc as bacc
    import concourse.mybir as mybir
    import concourse.tile as tile

    fp32 = mybir.dt.float32
    bf16 = mybir.dt.bfloat16
    RELU = mybir.ActivationFunctionType.Relu
    ABS = mybir.ActivationFunctionType.Abs
    ADD = mybir.AluOpType.add
    X = mybir.AxisListType.X

    nc = bacc.Bacc("TRN2", target_bir_lowering=False, debug=False,
                   num_devices=N_CORES)
    xT_d = nc.dram_tensor("xT", [128, B], bf16, kind="ExternalInput")
    g1s0_d = nc.dram_tensor("g1s0", [128, 256], bf16, kind="ExternalInput")
    g1b1_d = nc.dram_tensor("g1b1", [128, 3 * 256], bf16, kind="ExternalInput")
    g1b2_d = nc.dram_tensor("g1b2", [128, 4 * 256], bf16, kind="ExternalInput")
    w2s0_d = nc.dram_tensor("w2s0", [128, 4 * M], bf16, kind="ExternalInput")
    w2b1_d = nc.dram_tensor("w2b1", [128, 3 * 4 * M], bf16, kind="ExternalInput")
    w2b2_d = nc.dram_tensor("w2b2", [128, 4 * 4 * M], bf16, kind="ExternalInput")
    w3s0_d = nc.dram_tensor("w3s0", [128, tmax], bf16, kind="ExternalInput")
    w3b1_d = nc.dram_tensor("w3b1", [128, 3 * tmax], bf16, kind="ExternalInput")
    w3b2_d = nc.dram_tensor("w3b2", [128, 4 * tmax], bf16, kind="ExternalInput")
    xr_d = nc.dram_tensor("xrall", [3, JN * B], bf16, kind="ExternalInput")
    out_d = nc.dram_tensor("out", [B, JN], fp32, kind="ExternalOutput")

    with tile.TileContext(nc) as tc:
        with (
            tc.tile_pool(name="const", bufs=1) as const,
            tc.tile_pool(name="act", bufs=2) as apool,
            tc.tile_pool(name="small", bufs=1) as spool,
            tc.tile_pool(name="psa", bufs=2, space="PSUM") as pspool,
            tc.tile_pool(name="psb", bufs=2, space="PSUM") as ps3pool,
        ):
            # static SBUF tiles
            xT_t = const.tile([128, B], bf16, tag="xT")
            g1all = const.tile([128, JN * 256], bf16, tag="g1all")
            w2all = const.tile([128, JN * 4 * M], bf16, tag="w2all")
            w3all = const.tile([128, JN * tmax], bf16, tag="w3all")
            repall = const.tile([128, JN * B], bf16, tag="repall")
            c2 = spool.tile([128, 128], fp32, tag="c2")

            # ---- weight prefetch: 3 queues, contiguous sources, ordered
            # by first use. gpsimd's SWDGE is free here (descriptor
            # generation happens before any compute contends for SBUF).
            # Queue facts (measured): the sync HWDGE ring delivers only
            # ~8-10 GB/s here no matter the shape; the scalar HWDGE ring
            # does 60-85 GB/s; gpsimd SWDGE 35-130 GB/s. So: sync gets
            # only xT (first, small, needed at ~10us), scalar gets all
            # [128, .] weights, SWDGE gets the odd-shaped w3 [67, .] and
            # xr [3, .] (HWDGE serializes those onto one SDMA engine).
            nc.sync.dma_start(xT_t[:], xT_d.ap())
            nc.scalar.dma_start(g1all[:, 0:256], g1s0_d.ap())
            nc.vector.memset(repall[64:128, :], 0)
            nc.gpsimd.dma_start(repall[64:67, :], xr_d.ap())
            nc.scalar.dma_start(w2all[:, 0 : 4 * M], w2s0_d.ap())
            nc.gpsimd.dma_start(w3all[:, 0:tmax], w3s0_d.ap())
            nc.scalar.dma_start(g1all[:, 256 : 4 * 256], g1b1_d.ap())
            nc.gpsimd.dma_start(w3all[:, tmax : 4 * tmax], w3b1_d.ap())
            nc.scalar.dma_start(w2all[:, 4 * M : 4 * 4 * M], w2b1_d.ap())
            nc.scalar.dma_start(g1all[:, 4 * 256 :], g1b2_d.ap())
            nc.gpsimd.dma_start(w3all[:, 4 * tmax :], w3b2_d.ap())
            nc.scalar.dma_start(w2all[:, 4 * 4 * M :], w2b2_d.ap())

            h1T_t = {}

            # PE warm-up: dummy matmul burst while the DMAs land. The HAM
            # clock gate opens (K=8/8, 2.4 GHz) after ~3.4us of sustained
            # PE busy; without this the whole first half runs at 1.2 GHz.
            # memset on Vector (idle), psum from the ps3 pool (first real
            # use is node 0's L3, ~2 slots in).
            warm_n = int(_os.environ.get("CTP_WARM", "8"))
            warm_s = spool.tile([64, 768], bf16, tag="warm")
            warm_ps = None
            if warm_n:
                nc.vector.memset(warm_s[:], 0)
                warm_ps = ps3pool.tile([128, 512], fp32, tag="ps3",
                                       name="warm_ps")
                for _ in range(warm_n):
                    nc.tensor.matmul(warm_ps[:], warm_s[:, 0:128],
                                     warm_s[:, 256:768], start=True, stop=True)

            def filler(k):
                # fill-phase PE keep-alive: the pipeline isn't full yet, so
                # the PE would idle waiting on the ACT chain / psum-buffer
                # recycling; any idle window re-throttles the HAM clock
                # gate to 1.2 GHz and the cold slots then take ~9us instead
                # of ~5.7 (it takes until ~t=58us to re-lock 2.4 GHz).
                # Dummy FD=512 matmuls in the FIFO bridge those known gaps.
                # FD=128: ~57ns warm / ~107ns cold each, so a misjudged
                # count displaces little real work.
                for _ in range(k):
                    if warm_ps is not None:
                        nc.tensor.matmul(warm_ps[:, 0:128], warm_s[:, 0:128],
                                         warm_s[:, 256:384], start=True,
                                         stop=True)

            def emit_l1_unit(n, pair):
                # two K=64 matmuls run concurrently in row groups (0,0) and
                # (64,0): h-chunk 2*pair in array rows 0:64, 2*pair+1 in
                # rows 64:128 (g1/xT are laid out accordingly on the host).
                g1_t = g1all[:, n * 256 : (n + 1) * 256]
                h1T = h1T_t[n]
                psA = pspool.tile([128, B], fp32, tag="ps", name=f"psA_{n}_{pair}")
                psB = pspool.tile([128, B], fp32, tag="ps", name=f"psB_{n}_{pair}")

                def mmA(bc2):
                    nc.tensor.matmul(
                        psA[:, bc2 * 512 : (bc2 + 1) * 512],
                        g1_t[0:64, pair * 128 : (pair + 1) * 128],
                        xT_t[0:64, bc2 * 512 : (bc2 + 1) * 512],
                        start=True, stop=True, tile_position=(0, 0),
                    )

                def mmB(bc2):
                    nc.tensor.matmul(
                        psB[:, bc2 * 512 : (bc2 + 1) * 512],
                        g1_t[64:128, pair * 128 : (pair + 1) * 128],
                        xT_t[64:128, bc2 * 512 : (bc2 + 1) * 512],
                        start=True, stop=True, tile_position=(64, 0),
                    )
                # A0,B0,B1,A1: the redundant 2nd LDWEIGHTS of each row
                # group waits for that group's 1st matmul to finish; this
                # order lets it load while the OTHER group streams, so the
                # two halves of each pair overlap fully.
                mmA(0); mmB(0); mmB(1); mmA(1)
                nc.scalar.activation(
                    h1T[:, (2 * pair) * B : (2 * pair + 1) * B], psA[:], RELU)
                if pair == 1 and _os.environ.get("CTP_C3S", "0") == "1":
                    # split chunk3's ACT into bc halves: L2's hc3 matmuls
                    # (the slot's chain terminal) can issue after the bc0
                    # half instead of idling ACT ~0.65us until the full
                    # [128,1024] copy completes.
                    c3 = (2 * pair + 1) * B
                    nc.scalar.activation(
                        h1T[:, c3 : c3 + 512], psB[:, 0:512], RELU)
                    nc.scalar.activation(
                        h1T[:, c3 + 512 : c3 + B], psB[:, 512:B], RELU)
                else:
                    nc.scalar.activation(
                        h1T[:, (2 * pair + 1) * B : (2 * pair + 2) * B],
                        psB[:], RELU)

            colt = _os.environ.get("CTP_COLT", "1") == "1"

            def emit_l2_mm(n, bc2, ps2, hcs):
                # default: both batch halves accumulate in ONE [64,1024]
                # tile (cols 0:512 = bc half 0 in bank k, 512:1024 = half
                # 1 in bank k+1) so r1 is a single fused ACT.
                # CTP_COLT=1: half 1 instead goes to PE column group
                # (0,64) -> psum partitions 64:128, same columns; the two
                # 4-chunk accumulations then stream CONCURRENTLY through
                # the PE (measured 2.38x for col-tiling) halving L2's PE
                # time, at the cost of two r1 ACTs (one partition-shifted).
                w2_t = w2all[:, n * 4 * M : (n + 1) * 4 * M]
                h1T = h1T_t[n]
                for hc in hcs:
                    if colt and bc2 == 1:
                        out = ps2[64:128, 0:512]
                        kw = {"tile_position": (0, 64)}
                    else:
                        out = ps2[0:64, bc2 * 512 : (bc2 + 1) * 512]
                        kw = {"tile_position": (0, 0)} if colt else {}
                    nc.tensor.matmul(
                        out,
                        w2_t[:, hc * M : (hc + 1) * M],
                        h1T[:, hc * B + bc2 * 512 : hc * B + (bc2 + 1) * 512],
                        start=(hc == 0), stop=(hc == 3), **kw,
                    )

            def emit_l2_act(n, ps2):
                if colt:
                    nc.scalar.activation(
                        repall[0:64, n * B : n * B + 512],
                        ps2[0:64, 0:512], RELU)
                    nc.scalar.activation(
                        repall[0:64, n * B + 512 : (n + 1) * B],
                        ps2[64:128, 0:512], RELU)
                else:
                    nc.scalar.activation(
                        repall[0:64, n * B : (n + 1) * B], ps2[0:64, :], RELU)

            abs_scratch = spool.tile([128, 320], bf16, tag="abs_scr")

            def emit_l3_unit(n, J, bc, on_act=False):
                w3_t = w3all[:, n * tmax : (n + 1) * tmax]
                rep = repall[:, n * B : (n + 1) * B]
                ps3 = ps3pool.tile([128, B], fp32, tag="ps3")
                stat = rep[:, bc * 128 : (bc + 1) * 128]
                nc.tensor.matmul(ps3[:, 0:512], stat, w3_t[:, 0:512],
                                 start=True, stop=True)
                nc.tensor.matmul(ps3[:, 512 : t[J]], stat, w3_t[:, 512 : t[J]],
                                 start=True, stop=True)
                col = bc * 8 + J
                if on_act:
                    # drain-step offload: ACT is idle, DVE is the drain
                    # bottleneck; Abs+accum_out computes the window sums.
                    wj = t[J] // 2
                    for s in range(2):
                        nc.scalar.activation(
                            abs_scratch[:, 0:wj],
                            ps3[:, s * wj : (s + 1) * wj], ABS,
                            accum_out=c2[:, 2 * col + s : 2 * col + s + 1])
                else:
                    nc.vector.tensor_reduce(
                        c2[:, 2 * col : 2 * col + 2],
                        ps3[:, 0 : t[J]].rearrange("p (s w) -> p s w", s=2),
                        axis=X, op=ADD, apply_absolute_value=True)

            fin_split = _os.environ.get("CTP_FIN", "1") == "1"
            gtt = _os.environ.get("CTP_GTT", "1") == "1"

            def emit_final(cols, osb_tag):
                # out[:, j] = relu(0.5*(cpos - cneg)) for osb col range
                lo, hi = cols
                c3 = c2[:, 2 * lo : 2 * hi].rearrange("p (c s) -> p c s", s=2)
                t1 = spool.tile([128, hi - lo], fp32, tag=osb_tag + "t")
                eng = nc.gpsimd if gtt else nc.vector
                eng.tensor_tensor(t1[:], c3[:, :, 0], c3[:, :, 1],
                                  op=mybir.AluOpType.subtract)
                osb = spool.tile([128, hi - lo], fp32, tag=osb_tag)
                nc.scalar.activation(osb[:], t1[:], RELU, scale=0.5)
                # SWDGE: the sync ring would take ~2us for these 16KB
                nc.gpsimd.dma_start(
                    out_d.ap().rearrange("(k p) n -> p k n", p=128)[:, lo // 8 : hi // 8, :],
                    osb[:].rearrange("p (k n) -> p k n", n=JN),
                )

            # 3-stage software pipeline: step k = L1(k) + L2(k-1) + L3(k-2).
            # Every stage consumes data finished a full step earlier, so no
            # matmul waits on a same-step ACT: the once-per-slot ~0.5us PE
            # stall of the 2-stage version re-throttled the HAM clock gate
            # (K=4/8 for 2 of every 3 windows -> PE at 1.2 GHz half the
            # time despite >93% busy).
            dof = _os.environ.get("CTP_DOF", "1") == "1"
            pipe2 = _os.environ.get("CTP_PIPE", "3") == "2"
            n_steps = JN + 1 if pipe2 else JN + 2
            for step in range(n_steps):
                if pipe2:
                    nxt = step if step < JN else None
                    cur = nxt
                    prev = step - 1 if step > 0 else None
                else:
                    nxt = step if step < JN else None
                    cur = step - 1 if 1 <= step <= JN else None
                    prev = step - 2 if step >= 2 else None
                if nxt is not None:
                    h1T_t[nxt] = apool.tile([128, 4 * B], bf16, tag="h1",
                                            name=f"h1T_{nxt}")
                drain_act = nxt is None and cur is None and dof

                def l3(k):
                    if prev is not None:
                        emit_l3_unit(prev, prev, k,
                                     on_act=drain_act and k in (3, 5, 7))
                fill_n = int(_os.environ.get("CTP_FILLS", "0"))
                if step == 0:
                    fill_n = int(_os.environ.get("CTP_FILL0", "0"))
                elif step == 1 and not pipe2:
                    fill_n = int(_os.environ.get("CTP_FILL1", "0"))
                if nxt is not None:
                    emit_l1_unit(nxt, 0)
                filler(fill_n)
                l3(0); l3(1)
                if nxt is not None:
                    emit_l1_unit(nxt, 1)
                filler(fill_n)
                l3(2); l3(3)
                if fin_split and step == n_steps - 1:
                    # drain step: node 7's bc0-3 reduces just emitted, so
                    # osb cols 0:32 are complete; ship that half early.
                    emit_final((0, 32), "osb0")
                ps2 = None
                if cur is not None:
                    if colt:
                        ps2 = pspool.tile([128, 512], fp32, tag="ps",
                                          name=f"ps2_{step}")
                        # interleave the halves' chunks so the two column
                        # groups stream through the PE concurrently
                        for hc in range(2):
                            emit_l2_mm(cur, 0, ps2, [hc])
                            emit_l2_mm(cur, 1, ps2, [hc])
                    else:
                        ps2 = pspool.tile([64, B], fp32, tag="ps",
                                          name=f"ps2_{step}")
                        emit_l2_mm(cur, 0, ps2, range(4))
                l3(4); l3(5)
                if cur is not None:
                    if colt:
                        for hc in range(2, 4):
                            emit_l2_mm(cur, 0, ps2, [hc])
                            emit_l2_mm(cur, 1, ps2, [hc])
                    else:
                        emit_l2_mm(cur, 1, ps2, range(4))
                    emit_l2_act(cur, ps2)
                filler(fill_n)
                l3(6); l3(7)
                if cur is not None and not pipe2:
                    h1T_t.pop(cur, None)
                if pipe2 and prev is not None:
                    h1T_t.pop(prev, None)

            if fin_split:
                emit_final((32, 64), "osb1")
            else:
                emit_final((0, 64), "osb")
    nc.compile()
    return nc


def _get_program(w1, t, tmax):
    import os
    key = (w1, t, tmax, os.environ.get("CTP_WARM", ""),
           os.environ.get("CTP_FIN", ""), os.environ.get("CTP_DOF", ""),
           os.environ.get("CTP_C3S", ""), os.environ.get("CTP_GTT", ""),
           os.environ.get("CTP_PIPE", ""), os.environ.get("CTP_COLT", ""),
           os.environ.get("CTP_FILL0", ""), os.environ.get("CTP_FILL1", ""), os.environ.get("CTP_FILLS", ""))
    if key not in _PROGRAM_CACHE:
        _PROGRAM_CACHE[key] = _build_program(w1, t, tmax)
    return _PROGRAM_CACHE[key]


def kernel(x, W1, W2, W3, b3, W4, b4):
    import os
    from concourse.bass_utils import run_bass_kernel_spmd

    in_maps, assign, w1, t, tmax = _prep(x, W1, W2, W3, b3, W4, b4)
    nc = _get_program(w1, t, tmax)

    trace = os.environ.get("CTP_KERNEL_TRACE", "0") == "1"
    kwargs = {}
    if trace:
        import types
        sys.path.insert(0, "/root/.axon_site")
        from trn_agent_boot.trn_boot import _ntff_profile_via_ctypes
        hook = _ntff_profile_via_ctypes("/opt/axon/libaxon_pjrt.so")
        mod = types.ModuleType("antenv.axon_hooks")
        mod.get_axon_ntff_profile_hook = lambda: hook
        mod.set_axon_ntff_profile_hook = lambda h: None
        sys.modules["antenv.axon_hooks"] = mod
        import concourse.bass_utils as bu
        bu.upload_artifacts = lambda tmpdir: f"local:{tmpdir}"
        tdir = os.environ.get("CTP_TRACE_DIR", "/tmp/ctp_trace")
        os.makedirs(tdir, exist_ok=True)
        kwargs = {"trace": True, "tmpdir": tdir}

    res = run_bass_kernel_spmd(nc, in_maps, list(range(N_CORES)), **kwargs)
    if trace:
        print(f"HW exec time: {res.exec_time_ns} ns")

    out = np.zeros((B, N_NODES), np.float32)
    for j in range(N_CORES):
        oj = np.asarray(res.results[j]["out"], np.float32)   # (B, JN)
        for J in range(JN):
            out[:, int(assign[J, j])] = oj[:, J]
    return out


# revision 26
# speedup vs baseline: 1.2914x; 1.0258x over previous
"""Trainium2 Bass kernel for nn_CausalTrajectoryPrediction.

Math (per node n of 64, batch B=1024):
    h1 = relu(x_masked @ W1[n].T)          x_masked = x with col n zeroed
    r1 = relu(h1 @ W2[n].T)
    h3 = relu([r1, x_n] @ W3[n].T + b3[n])
    out[:, n] = relu(h3 @ W4[n] + b4[n])

Restructuring (validated vs the fp32 reference on CPU):
  - The input mask folds into the weights on the host (zero the diagonal
    column of W1[n]); the "own value" path of W3 collapses to one column;
    b3 becomes a ones-row of the layer-3 stationary operand.
  - The last layer is eliminated via w*relu(z) = 0.5*(w*z + w*|z|):
        out[:, n] = 0.5 * relu(c_pos - c_neg)
    where c_pos/c_neg are DVE abs-reduces over two fixed windows of the
    layer-3 PSUM row. Columns are pre-scaled by |W4| and grouped by
    sign(W4) on the host. The linear term a = rep @ (W3ext@W4 + 2*b4*e1)
    is carried by two extra nonnegative columns (v+ in the positive
    window, v- in the negative window; rep is made elementwise >= 0 by
    splitting x_n into x+/x-), so no extra matmul or PSUM tile is needed.
  - All 8 cores share one program, but the sign-split point differs per
    node. Nodes are assigned to program slots sorted by split point; the
    few "middle" columns that are positive on some cores and negative on
    others sit inside the positive window and are duplicated at the tail
    with weight 2 (|z| - 2|z| = -|z|) on cores where they are negative.
  - Sharding: 8 nodes per core (weights are NOT replicated -> 8x less
    HBM traffic), full batch per core. Host gathers (1024, 8) per core.

v3 changes vs v2 (trace-driven, see HW model below):
  - DMA prefetch: every transfer now reads a CONTIGUOUS DRAM tensor
    (v2 sliced wide tensors; the strided reads landed all descriptors on
    ONE of the 16 SDMA engines -> 25 GB/s, weights not resident until
    t=38us, PE stalling + HAM re-throttling to 1.2 GHz mid-kernel).
    Transfers are spread over three queues (sync/scalar HWDGE + gpsimd
    SWDGE - idle before compute starts) and ordered by first use.
  - Warm-up scratch memset moved to the (otherwise idle) Vector engine
    and the warm psum to the ps3 pool, so the dummy-matmul burst starts
    right after the engine preamble (~5us) instead of ~8us, and the PE's
    HAM clock gate (default K=4/8 = 1.2 GHz; K=8/8 = 2.4 GHz after
    ~3.4us of sustained busy) is fully open when the real stream begins.
  - L2 accumulates both batch halves into ONE [64,1024] psum tile
    (different banks) and r1 is a single fused [64,1024] ACT: saves one
    352-cycle ACT instruction start per node on the critical engine.
  - Drain step: half the last node's window reduces run on ACT
    (abs+accum_out) since ACT is idle there; final combine subtract on
    gpsimd; output DMA split in halves (ship cols 0:32 early).

HW model (measured on this kernel's trace):
  ACT ACTIVATE dur ~ 304 + FD ns; DVE TENSOR_REDUCE dur ~ 307 + 1.04*FD
  (PSUM fp32 src is locked to 1 elem/cycle on both engines; GPSIMD has
  no PSUM port). Per node: ACT = 4 h1-chunk relus [128,1024] + fused r1
  = ~6.6us, DVE = 8 window-reduces [128,~550] = ~7.2us, PE (warm,
  2.4GHz) = L1 ~1.0 + L2 ~2.1 + L3 ~3.2 = ~6.3us. The slot is EW-bound
  at ~7us; everything else (DMA, PE clock, fill/drain) must stay off
  the critical path. Framework overhead is ~7.3us preamble (engine
  TENSOR_LOADs + barriers before the first DMA doorbell) + ~9.5us
  postamble (semaphore teardown) and is not controllable from here.

Set CTP_KERNEL_TRACE=1 to capture a neuron-profile trace and print
"HW exec time: <ns> ns".
"""
import sys

sys.path.insert(0, "/opt/trn_rl_repo")

import numpy as np
import ml_dtypes

N_NODES = 64
H = 512
B = 1024
M = 64
N_CORES = 8
JN = 8           # nodes (slots) per core
BC = 8           # batch chunks of 128
BF16 = ml_dtypes.bfloat16

_PROGRAM_CACHE = {}


def _prep(x, W1, W2, W3, b3, W4, b4):
    """Build per-core input maps + program-shape metadata."""
    x = np.asarray(x, np.float32)
    W1 = np.asarray(W1, np.float32)
    W2 = np.asarray(W2, np.float32)
    W3 = np.asarray(W3, np.float32)
    b3 = np.asarray(b3, np.float32)
    W4 = np.asarray(W4, np.float32)
    b4 = np.asarray(b4, np.float32)

    ppos = (W4 >= 0).sum(axis=1)            # sign-split point per node
    order = np.argsort(ppos, kind="stable")
    assign = order.reshape(JN, N_CORES)     # assign[J, core] -> node id
    pmax = np.array([int(ppos[assign[J]].max()) for J in range(JN)])
    pmin = np.array([int(ppos[assign[J]].min()) for J in range(JN)])
    assert pmax.max() < 512, "degenerate all-positive W4 row not supported"
    # symmetric reduce windows: [0:w1) abs+, [w1:2*w1) abs- (zero padded).
    w1 = np.maximum(1 + pmax, 513 - pmin)
    t = 2 * w1                              # per-slot moving width
    # Slot order is arbitrary: run the widest-t group first (its extra
    # reduce width overlaps pipeline fill) and the narrowest last (the
    # drain step's 8 serial reduces scale with t of the final slot).
    perm = np.argsort(-t, kind="stable")
    assign = assign[perm]
    w1 = w1[perm]
    t = t[perm]
    pmax = pmax[perm]
    pmin = pmin[perm]
    tmax = int(t.max())

    xT1 = np.ascontiguousarray(x.T).astype(BF16)         # (64, 1024)
    xT = np.concatenate([xT1, xT1], axis=0)              # (128, 1024) doubled
    in_maps = []
    for j in range(N_CORES):
        g1 = np.zeros((JN, 128, 256), BF16)
        w2 = np.zeros((JN, 128, 4 * M), BF16)
        w3 = np.zeros((JN, 128, tmax), BF16)
        xr = np.zeros((JN, 3, B), BF16)
        for J in range(JN):
            n = int(assign[J, j])
            P = int(ppos[n])
            g1t = W1[n].T.copy()                          # (64 i, 512 h)
            g1t[n, :] = 0.0
            for pair in range(2):
                g1[J][0:64, pair * 128 : (pair + 1) * 128] = \
                    g1t[:, (2 * pair) * 128 : (2 * pair + 1) * 128].astype(BF16)
                g1[J][64:128, pair * 128 : (pair + 1) * 128] = \
                    g1t[:, (2 * pair + 1) * 128 : (2 * pair + 2) * 128].astype(BF16)
            w2t = W2[n].T                                 # (512 h, 64 m)
            w2[J] = np.ascontiguousarray(
                w2t.reshape(4, 128, M).transpose(1, 0, 2).reshape(128, 4 * M)
            ).astype(BF16)

            w4 = W4[n]
            w3ext = np.zeros((66, H), np.float32)
            w3ext[:64] = W3[n, :, :64].T
            w3ext[64] = W3[n, :, 64 + n]
            w3ext[65] = b3[n]
            scaled = w3ext * np.abs(w4)[None, :]
            pos = np.where(w4 >= 0)[0]
            neg = np.where(w4 < 0)[0]
            nmid = pmax[J] - P
            midc, certain = neg[:nmid], neg[nmid:]

            def lift(c):                                  # (66,k) -> (67,k)
                o = np.zeros((67, c.shape[1]), np.float32)
                o[:64] = c[:64]
                o[64] = c[64]
                o[65] = -c[64]
                o[66] = c[65]
                return o

            v = (w3ext @ w4).astype(np.float32)
            v[65] += 2.0 * b4[n]
            v67 = np.zeros(67, np.float32)
            v67[:64] = v[:64]
            v67[64] = v[64]
            v67[65] = -v[64]
            v67[66] = v[65]

            pad = np.zeros((67, tmax), np.float32)
            pad[:, 0] = np.maximum(v67, 0)                       # colA
            pad[:, 1 : 1 + P] = lift(scaled[:, pos])
            pad[:, 1 + P : 1 + pmax[J]] = lift(scaled[:, midc])
            nc_ = len(certain)
            pad[:, w1[J] : w1[J] + nc_] = lift(scaled[:, certain])
            pad[:, w1[J] + nc_ : w1[J] + nc_ + nmid] = 2.0 * lift(scaled[:, midc])
            pad[:, w1[J] + nc_ + nmid] = np.maximum(-v67, 0)     # colB
            w3[J][:67] = pad.astype(BF16)

            xr[J, 0] = np.maximum(x[:, n], 0).astype(BF16)
            xr[J, 1] = np.maximum(-x[:, n], 0).astype(BF16)
            xr[J, 2] = 1.0
        # partition-major packing: node n at free-axis cols [n*w : (n+1)*w]
        g1p = np.ascontiguousarray(g1.transpose(1, 0, 2).reshape(128, JN * 256))
        w2p = np.ascontiguousarray(w2.transpose(1, 0, 2).reshape(128, JN * 4 * M))
        w3p = np.ascontiguousarray(w3.transpose(1, 0, 2).reshape(128, JN * tmax))
        xrp = np.ascontiguousarray(xr.transpose(1, 0, 2).reshape(3, JN * B))
        # every DMA below reads one of these CONTIGUOUS arrays end-to-end
        # (a strided DRAM read serializes onto one SDMA engine: ~25 GB/s)
        cc = lambda a: np.ascontiguousarray(a)
        in_maps.append({
            "xTa": np.ascontiguousarray(xT[:, :512]),
            "xTb": np.ascontiguousarray(xT[:, 512:]),
            "g1s0": cc(g1p[:, :256]),
            "g1b1": cc(g1p[:, 256 : 4 * 256]),
            "g1b2": cc(g1p[:, 4 * 256 :]),
            "w2s0": cc(w2p[:, : 4 * M]),
            "w2b1": cc(w2p[:, 4 * M : 4 * 4 * M]),
            "w2b2": cc(w2p[:, 4 * 4 * M :]),
            "w3s0": cc(w3p[:, :tmax]),
            "w3b1": cc(w3p[:, tmax : 4 * tmax]),
            "w3b2": cc(w3p[:, 4 * tmax :]),
            "xrall": xrp,
        })
    return in_maps, assign, tuple(int(v) for v in w1), tuple(int(v) for v in t), tmax


def _patch_ldw_opt():
    # walrus's LDWEIGHTS-dedup pass is off by default in bass_utils; our
    # L3 issues two matmuls per bc chunk with the SAME stationary, so the
    # redundant second load (~107ns x 64/core) is pure PE time. Enable the
    # pass (numerics are verified by the caller's rel-err check).
    import os as _os
    if _os.environ.get("CTP_LDWOPT", "0") != "1":
        return
    import concourse.bass_utils as _bu
    import inspect as _inspect
    srcf = _inspect.getsourcefile(_bu)
    # patch the module constant in memory only: run_command sees the list
    # built inside compile_bir_to_neff; easiest robust hook is wrapping
    # run_command to rewrite the flag.
    if getattr(_bu, "_ctp_ldw_patched", False):
        return
    _orig = _bu.run_command

    def _patched(cmd, *a, **kw):
        cmd = [c.replace("--enable-ldw-opt=false", "--enable-ldw-opt=true")
               if isinstance(c, str) else c for c in cmd]
        return _orig(cmd, *a, **kw)

    _bu.run_command = _patched
    _bu._ctp_ldw_patched = True


def _build_program(w1, t, tmax):
    import os as _os
    _patch_ldw_opt()
    import concourse.bacc as bacc
    import concourse.mybir as mybir
    import concourse.tile as tile

    fp32 = mybir.dt.float32
    bf16 = mybir.dt.bfloat16
    RELU = mybir.ActivationFunctionType.Relu
    ABS = mybir.ActivationFunctionType.Abs
    ADD = mybir.AluOpType.add
    X = mybir.AxisListType.X

    nc = bacc.Bacc("TRN2", target_bir_lowering=False, debug=False,
                   num_devices=N_CORES)
    xTa_d = nc.dram_tensor("xTa", [128, 512], bf16, kind="ExternalInput")
    xTb_d = nc.dram_tensor("xTb", [128, 512], bf16, kind="ExternalInput")
    g1s0_d = nc.dram_tensor("g1s0", [128, 256], bf16, kind="ExternalInput")
    g1b1_d = nc.dram_tensor("g1b1", [128, 3 * 256], bf16, kind="ExternalInput")
    g1b2_d = nc.dram_tensor("g1b2", [128, 4 * 256], bf16, kind="ExternalInput")
    w2s0_d = nc.dram_tensor("w2s0", [128, 4 * M], bf16, kind="ExternalInput")
    w2b1_d = nc.dram_tensor("w2b1", [128, 3 * 4 * M], bf16, kind="ExternalInput")
    w2b2_d = nc.dram_tensor("w2b2", [128, 4 * 4 * M], bf16, kind="ExternalInput")
    w3s0_d = nc.dram_tensor("w3s0", [128, tmax], bf16, kind="ExternalInput")
    w3b1_d = nc.dram_tensor("w3b1", [128, 3 * tmax], bf16, kind="ExternalInput")
    w3b2_d = nc.dram_tensor("w3b2", [128, 4 * tmax], bf16, kind="ExternalInput")
    xr_d = nc.dram_tensor("xrall", [3, JN * B], bf16, kind="ExternalInput")
    out_d = nc.dram_tensor("out", [B, JN], fp32, kind="ExternalOutput")

    with tile.TileContext(nc) as tc:
        with (
            tc.tile_pool(name="const", bufs=1) as const,
            tc.tile_pool(name="act", bufs=2) as apool,
            tc.tile_pool(name="small", bufs=1) as spool,
            tc.tile_pool(name="psa", bufs=2, space="PSUM") as pspool,
            tc.tile_pool(name="psb", bufs=2, space="PSUM") as ps3pool,
        ):
            # static SBUF tiles
            xT_t = const.tile([128, B], bf16, tag="xT")
            g1all = const.tile([128, JN * 256], bf16, tag="g1all")
            w2all = const.tile([128, JN * 4 * M], bf16, tag="w2all")
            w3all = const.tile([128, JN * tmax], bf16, tag="w3all")
            repall = const.tile([128, JN * B], bf16, tag="repall")
            c2 = spool.tile([128, 128], fp32, tag="c2")

            # ---- weight prefetch: 3 queues, contiguous sources, ordered
            # by first use. gpsimd's SWDGE is free here (descriptor
            # generation happens before any compute contends for SBUF).
            # Queue facts (measured): the sync HWDGE ring delivers only
            # ~8-10 GB/s here no matter the shape; the scalar HWDGE ring
            # does 60-85 GB/s; gpsimd SWDGE 35-130 GB/s. So: sync gets
            # only xT (first, small, needed at ~10us), scalar gets all
            # [128, .] weights, SWDGE gets the odd-shaped w3 [67, .] and
            # xr [3, .] (HWDGE serializes those onto one SDMA engine).
            nc.sync.dma_start(xT_t[:, 0:512], xTa_d.ap())
            nc.sync.dma_start(xT_t[:, 512:B], xTb_d.ap())
            nc.scalar.dma_start(g1all[:, 0:256], g1s0_d.ap())
            nc.vector.memset(repall[64:128, :], 0)
            nc.gpsimd.dma_start(repall[64:67, :], xr_d.ap())
            nc.scalar.dma_start(w2all[:, 0 : 4 * M], w2s0_d.ap())
            nc.gpsimd.dma_start(w3all[:, 0:tmax], w3s0_d.ap())
            nc.scalar.dma_start(g1all[:, 256 : 4 * 256], g1b1_d.ap())
            nc.gpsimd.dma_start(w3all[:, tmax : 4 * tmax], w3b1_d.ap())
            nc.scalar.dma_start(w2all[:, 4 * M : 4 * 4 * M], w2b1_d.ap())
            nc.scalar.dma_start(g1all[:, 4 * 256 :], g1b2_d.ap())
            nc.gpsimd.dma_start(w3all[:, 4 * tmax :], w3b2_d.ap())
            nc.scalar.dma_start(w2all[:, 4 * 4 * M :], w2b2_d.ap())

            h1T_t = {}

            # PE warm-up: dummy matmul burst while the DMAs land. The HAM
            # clock gate opens (K=8/8, 2.4 GHz) after ~3.4us of sustained
            # PE busy; without this the whole first half runs at 1.2 GHz.
            # memset on Vector (idle), psum from the ps3 pool (first real
            # use is node 0's L3, ~2 slots in).
            warm_n = int(_os.environ.get("CTP_WARM", "8"))
            warm_s = spool.tile([64, 768], bf16, tag="warm")
            warm_ps = None
            if warm_n:
                nc.vector.memset(warm_s[:], 0)
                warm_ps = ps3pool.tile([128, 512], fp32, tag="ps3",
                                       name="warm_ps")
                for _ in range(warm_n):
                    nc.tensor.matmul(warm_ps[:], warm_s[:, 0:128],
                                     warm_s[:, 256:768], start=True, stop=True)

            def filler(k):
                # fill-phase PE keep-alive: the pipeline isn't full yet, so
                # the PE would idle waiting on the ACT chain / psum-buffer
                # recycling; any idle window re-throttles the HAM clock
                # gate to 1.2 GHz and the cold slots then take ~9us instead
                # of ~5.7 (it takes until ~t=58us to re-lock 2.4 GHz).
                # Dummy FD=512 matmuls in the FIFO bridge those known gaps.
                # FD=128: ~57ns warm / ~107ns cold each, so a misjudged
                # count displaces little real work.
                for _ in range(k):
                    if warm_ps is not None:
                        nc.tensor.matmul(warm_ps[:, 0:128], warm_s[:, 0:128],
                                         warm_s[:, 256:384], start=True,
                                         stop=True)

            def emit_l1_unit(n, pair):
                # two K=64 matmuls run concurrently in row groups (0,0) and
                # (64,0): h-chunk 2*pair in array rows 0:64, 2*pair+1 in
                # rows 64:128 (g1/xT are laid out accordingly on the host).
                g1_t = g1all[:, n * 256 : (n + 1) * 256]
                h1T = h1T_t[n]
                psA = pspool.tile([128, B], fp32, tag="ps", name=f"psA_{n}_{pair}")
                psB = pspool.tile([128, B], fp32, tag="ps", name=f"psB_{n}_{pair}")

                def mmA(bc2):
                    nc.tensor.matmul(
                        psA[:, bc2 * 512 : (bc2 + 1) * 512],
                        g1_t[0:64, pair * 128 : (pair + 1) * 128],
                        xT_t[0:64, bc2 * 512 : (bc2 + 1) * 512],
                        start=True, stop=True, tile_position=(0, 0),
                    )

                def mmB(bc2):
                    nc.tensor.matmul(
                        psB[:, bc2 * 512 : (bc2 + 1) * 512],
                        g1_t[64:128, pair * 128 : (pair + 1) * 128],
                        xT_t[64:128, bc2 * 512 : (bc2 + 1) * 512],
                        start=True, stop=True, tile_position=(64, 0),
                    )
                # A0,B0,B1,A1: the redundant 2nd LDWEIGHTS of each row
                # group waits for that group's 1st matmul to finish; this
                # order lets it load while the OTHER group streams, so the
                # two halves of each pair overlap fully.
                mmA(0); mmB(0); mmB(1); mmA(1)
                nc.scalar.activation(
                    h1T[:, (2 * pair) * B : (2 * pair + 1) * B], psA[:], RELU)
                if pair == 1 and _os.environ.get("CTP_C3S", "0") == "1":
                    # split chunk3's ACT into bc halves: L2's hc3 matmuls
                    # (the slot's chain terminal) can issue after the bc0
                    # half instead of idling ACT ~0.65us until the full
                    # [128,1024] copy completes.
                    c3 = (2 * pair + 1) * B
                    nc.scalar.activation(
                        h1T[:, c3 : c3 + 512], psB[:, 0:512], RELU)
                    nc.scalar.activation(
                        h1T[:, c3 + 512 : c3 + B], psB[:, 512:B], RELU)
                else:
                    nc.scalar.activation(
                        h1T[:, (2 * pair + 1) * B : (2 * pair + 2) * B],
                        psB[:], RELU)

            colt = _os.environ.get("CTP_COLT", "1") == "1"

            def emit_l2_mm(n, bc2, ps2, hcs):
                # default: both batch halves accumulate in ONE [64,1024]
                # tile (cols 0:512 = bc half 0 in bank k, 512:1024 = half
                # 1 in bank k+1) so r1 is a single fused ACT.
                # CTP_COLT=1: half 1 instead goes to PE column group
                # (0,64) -> psum partitions 64:128, same columns; the two
                # 4-chunk accumulations then stream CONCURRENTLY through
                # the PE (measured 2.38x for col-tiling) halving L2's PE
                # time, at the cost of two r1 ACTs (one partition-shifted).
                w2_t = w2all[:, n * 4 * M : (n + 1) * 4 * M]
                h1T = h1T_t[n]
                for hc in hcs:
                    if colt and bc2 == 1:
                        out = ps2[64:128, 0:512]
                        kw = {"tile_position": (0, 64)}
                    else:
                        out = ps2[0:64, bc2 * 512 : (bc2 + 1) * 512]
                        kw = {"tile_position": (0, 0)} if colt else {}
                    nc.tensor.matmul(
                        out,
                        w2_t[:, hc * M : (hc + 1) * M],
                        h1T[:, hc * B + bc2 * 512 : hc * B + (bc2 + 1) * 512],
                        start=(hc == 0), stop=(hc == 3), **kw,
                    )

            def emit_l2_act(n, ps2):
                if colt:
                    nc.scalar.activation(
                        repall[0:64, n * B : n * B + 512],
                        ps2[0:64, 0:512], RELU)
                    nc.scalar.activation(
                        repall[0:64, n * B + 512 : (n + 1) * B],
                        ps2[64:128, 0:512], RELU)
                else:
                    nc.scalar.activation(
                        repall[0:64, n * B : (n + 1) * B], ps2[0:64, :], RELU)

            abs_scratch = spool.tile([128, 320], bf16, tag="abs_scr")

            def emit_l3_unit(n, J, bc, on_act=False):
                w3_t = w3all[:, n * tmax : (n + 1) * tmax]
                rep = repall[:, n * B : (n + 1) * B]
                ps3 = ps3pool.tile([128, B], fp32, tag="ps3")
                stat = rep[:, bc * 128 : (bc + 1) * 128]
                nc.tensor.matmul(ps3[:, 0:512], stat, w3_t[:, 0:512],
                                 start=True, stop=True)
                nc.tensor.matmul(ps3[:, 512 : t[J]], stat, w3_t[:, 512 : t[J]],
                                 start=True, stop=True)
                col = bc * 8 + J
                if on_act:
                    # drain-step offload: ACT is idle, DVE is the drain
                    # bottleneck; Abs+accum_out computes the window sums.
                    wj = t[J] // 2
                    for s in range(2):
                        nc.scalar.activation(
                            abs_scratch[:, 0:wj],
                            ps3[:, s * wj : (s + 1) * wj], ABS,
                            accum_out=c2[:, 2 * col + s : 2 * col + s + 1])
                else:
                    nc.vector.tensor_reduce(
                        c2[:, 2 * col : 2 * col + 2],
                        ps3[:, 0 : t[J]].rearrange("p (s w) -> p s w", s=2),
                        axis=X, op=ADD, apply_absolute_value=True)

            fin_split = _os.environ.get("CTP_FIN", "1") == "1"
            gtt = _os.environ.get("CTP_GTT", "1") == "1"

            def emit_final(cols, osb_tag):
                # out[:, j] = relu(0.5*(cpos - cneg)) for osb col range
                lo, hi = cols
                c3 = c2[:, 2 * lo : 2 * hi].rearrange("p (c s) -> p c s", s=2)
                t1 = spool.tile([128, hi - lo], fp32, tag=osb_tag + "t")
                eng = nc.gpsimd if gtt else nc.vector
                eng.tensor_tensor(t1[:], c3[:, :, 0], c3[:, :, 1],
                                  op=mybir.AluOpType.subtract)
                osb = spool.tile([128, hi - lo], fp32, tag=osb_tag)
                nc.scalar.activation(osb[:], t1[:], RELU, scale=0.5)
                # SWDGE: the sync ring would take ~2us for these 16KB
                nc.gpsimd.dma_start(
                    out_d.ap().rearrange("(k p) n -> p k n", p=128)[:, lo // 8 : hi // 8, :],
                    osb[:].rearrange("p (k n) -> p k n", n=JN),
                )

            # 3-stage software pipeline: step k = L1(k) + L2(k-1) + L3(k-2).
            # Every stage consumes data finished a full step earlier, so no
            # matmul waits on a same-step ACT: the once-per-slot ~0.5us PE
            # stall of the 2-stage version re-throttled the HAM clock gate
            # (K=4/8 for 2 of every 3 windows -> PE at 1.2 GHz half the
            # time despite >93% busy).
            dof = _os.environ.get("CTP_DOF", "1") == "1"
            pipe2 = _os.environ.get("CTP_PIPE", "3") == "2"
            n_steps = JN + 1 if pipe2 else JN + 2
            for step in range(n_steps):
                if pipe2:
                    nxt = step if step < JN else None
                    cur = nxt
                    prev = step - 1 if step > 0 else None
                else:
                    nxt = step if step < JN else None
                    cur = step - 1 if 1 <= step <= JN else None
                    prev = step - 2 if step >= 2 else None
                if nxt is not None:
                    h1T_t[nxt] = apool.tile([128, 4 * B], bf16, tag="h1",
                                            name=f"h1T_{nxt}")
                drain_act = nxt is None and cur is None and dof

                def l3(k):
                    if prev is not None:
                        emit_l3_unit(prev, prev, k,
                                     on_act=drain_act and k in (5, 7))
                fill_n = int(_os.environ.get("CTP_FILLS", "0"))
                if step == 0:
                    fill_n = int(_os.environ.get("CTP_FILL0", "0"))
                elif step == 1 and not pipe2:
                    fill_n = int(_os.environ.get("CTP_FILL1", "0"))
                if nxt is not None:
                    emit_l1_unit(nxt, 0)
                filler(fill_n)
                l3(0); l3(1)
                if nxt is not None:
                    emit_l1_unit(nxt, 1)
                filler(fill_n)
                l3(2); l3(3)
                if fin_split and step == n_steps - 1:
                    # drain step: node 7's bc0-3 reduces just emitted, so
                    # osb cols 0:32 are complete; ship that half early.
                    emit_final((0, 32), "osb0")
                ps2 = None
                if cur is not None:
                    if colt:
                        ps2 = pspool.tile([128, 512], fp32, tag="ps",
                                          name=f"ps2_{step}")
                        # interleave the halves' chunks so the two column
                        # groups stream through the PE concurrently
                        for hc in range(2):
                            emit_l2_mm(cur, 0, ps2, [hc])
                            emit_l2_mm(cur, 1, ps2, [hc])
                    else:
                        ps2 = pspool.tile([64, B], fp32, tag="ps",
                                          name=f"ps2_{step}")
                        emit_l2_mm(cur, 0, ps2, range(4))
                l3(4); l3(5)
                if cur is not None:
                    if colt:
                        for hc in range(2, 4):
                            emit_l2_mm(cur, 0, ps2, [hc])
                            emit_l2_mm(cur, 1, ps2, [hc])
                    else:
                        emit_l2_mm(cur, 1, ps2, range(4))
                    emit_l2_act(cur, ps2)
                filler(fill_n)
                l3(6); l3(7)
                if cur is not None and not pipe2:
                    h1T_t.pop(cur, None)
                if pipe2 and prev is not None:
                    h1T_t.pop(prev, None)

            if fin_split:
                emit_final((32, 64), "osb1")
            else:
                emit_final((0, 64), "osb")
    nc.compile()
    return nc


def _get_program(w1, t, tmax):
    import os
    key = (w1, t, tmax, os.environ.get("CTP_WARM", ""),
           os.environ.get("CTP_FIN", ""), os.environ.get("CTP_DOF", ""),
           os.environ.get("CTP_C3S", ""), os.environ.get("CTP_GTT", ""),
           os.environ.get("CTP_PIPE", ""), os.environ.get("CTP_COLT", ""),
           os.environ.get("CTP_FILL0", ""), os.environ.get("CTP_FILL1", ""), os.environ.get("CTP_FILLS", ""))
    if key not in _PROGRAM_CACHE:
        _PROGRAM_CACHE[key] = _build_program(w1, t, tmax)
    return _PROGRAM_CACHE[key]


def kernel(x, W1, W2, W3, b3, W4, b4):
    import os
    from concourse.bass_utils import run_bass_kernel_spmd

    in_maps, assign, w1, t, tmax = _prep(x, W1, W2, W3, b3, W4, b4)
    nc = _get_program(w1, t, tmax)

    trace = os.environ.get("CTP_KERNEL_TRACE", "0") == "1"
    kwargs = {}
    if trace:
        import types
        sys.path.insert(0, "/root/.axon_site")
        from trn_agent_boot.trn_boot import _ntff_profile_via_ctypes
        hook = _ntff_profile_via_ctypes("/opt/axon/libaxon_pjrt.so")
        mod = types.ModuleType("antenv.axon_hooks")
        mod.get_axon_ntff_profile_hook = lambda: hook
        mod.set_axon_ntff_profile_hook = lambda h: None
        sys.modules["antenv.axon_hooks"] = mod
        import concourse.bass_utils as bu
        bu.upload_artifacts = lambda tmpdir: f"local:{tmpdir}"
        tdir = os.environ.get("CTP_TRACE_DIR", "/tmp/ctp_trace")
        os.makedirs(tdir, exist_ok=True)
        kwargs = {"trace": True, "tmpdir": tdir}

    res = run_bass_kernel_spmd(nc, in_maps, list(range(N_CORES)), **kwargs)
    if trace:
        print(f"HW exec time: {res.exec_time_ns} ns")

    out = np.zeros((B, N_NODES), np.float32)
    for j in range(N_CORES):
        oj = np.asarray(res.results[j]["out"], np.float32)   # (B, JN)
        for J in range(JN):
            out[:, int(assign[J, j])] = oj[:, J]
    return out
